# revision 1
# baseline (speedup 1.0000x reference)
"""CrossFeatureAttention TRN2 kernel.

Full inputs -> full output. Sharding: data-parallel over (batch b, half of N1)
across 8 cores; each core computes out[b, h*2048:(h+1)*2048, :].

Math (per core, x1 slice q=2048 rows, x2[b] k=4096 rows, C=512):
    Q  = x1 @ Wq^T + bq
    K  = x2 @ Wk^T + bk
    V  = x2 @ Wv^T + bv
    P  = softmax(Q K^T / sqrt(C))          (no max subtraction; scores are small)
    out = (Q + P V) @ Wo^T + bo
        = x1 @ (Wo Wq)^T + (P V) @ Wo^T + (Wo bq + bo)     <- residual folded

The x1 @ (Wo Wq)^T term carries almost all of the output magnitude and runs in
fp32r; the attention path runs in bf16.  Attention is computed transposed
(S^T[k,q] = sum_c K^T[c,k] Q^T[c,q]) so exp(S^T) is already in the layout the
A^T matmul needs, and row sums come from a ones-matmul over partitions.
"""

import os
import sys

import numpy as np

for _p in ("/root/.axon_site", "/root/.axon_site/_ro/trn_rl_repo",
           "/root/.axon_site/_ro/pypackages"):
    if _p not in sys.path and os.path.isdir(_p):
        sys.path.append(_p)

import ml_dtypes

import concourse.bacc as bacc
import concourse.mybir as mybir
import concourse.tile as tile
from concourse import bass_isa, library_config, masks
from concourse.bass_utils import run_bass_kernel_spmd

F32 = mybir.dt.float32
F32R = mybir.dt.float32r
BF16 = mybir.dt.bfloat16
AF = mybir.ActivationFunctionType

B, N1, N2, C = 4, 4096, 4096, 512
NCORES = 8
QROWS = N1 * B // NCORES          # 2048 q rows per core
QC = 512                          # q-chunk (columns of S^T tiles)
NQC = QROWS // QC                 # 4 chunks
KT = N2 // 128                    # 32 k-tiles
CCH = C // 128                    # 4 contraction chunks
SCALE = 1.0 / float(np.sqrt(C))

_BUILT = None


def build():
    nc = bacc.Bacc(None, target_bir_lowering=False, debug=False)

    x1f_d = nc.dram_tensor("x1f", [QROWS, C], F32, kind="ExternalInput")
    x1b_d = nc.dram_tensor("x1b", [QROWS, C], BF16, kind="ExternalInput")
    x2b_d = nc.dram_tensor("x2b", [N2, C], BF16, kind="ExternalInput")
    wq_d = nc.dram_tensor("wq_t", [C, C], BF16, kind="ExternalInput")
    wk_d = nc.dram_tensor("wk_t", [C, C], BF16, kind="ExternalInput")
    wv_d = nc.dram_tensor("wv_t", [C, C], BF16, kind="ExternalInput")
    wo_d = nc.dram_tensor("wo_t", [C, C], BF16, kind="ExternalInput")
    wqo_d = nc.dram_tensor("wqo_t", [C, C], F32, kind="ExternalInput")
    bq_d = nc.dram_tensor("bq", [C], F32, kind="ExternalInput")
    bk_d = nc.dram_tensor("bk", [C], F32, kind="ExternalInput")
    bv_d = nc.dram_tensor("bv", [C], F32, kind="ExternalInput")
    bo2_d = nc.dram_tensor("bo2", [C], F32, kind="ExternalInput")
    out_d = nc.dram_tensor("out", [QROWS, C], F32, kind="ExternalOutput")

    with tile.TileContext(nc) as tc:
        with tc.tile_pool(name="cst", bufs=1) as cst, \
             tc.tile_pool(name="per", bufs=1) as per, \
             tc.tile_pool(name="sb", bufs=1) as sb, \
             tc.tile_pool(name="ps", bufs=1, space="PSUM") as ps:

            # ---- constants / weights ----
            ident = cst.tile([128, 128], F32)
            masks.make_identity(nc, ident[:])
            ones_bf = cst.tile([128, 128], BF16)
            nc.gpsimd.memset(ones_bf[:], 1.0)

            def load_w_bf(dram, nm):
                ts = []
                for cc in range(CCH):
                    t = cst.tile([128, C], BF16, name=f"{nm}{cc}", tag=f"{nm}{cc}")
                    nc.sync.dma_start(out=t[:], in_=dram[cc * 128:(cc + 1) * 128, :])
                    ts.append(t)
                return ts

            wk_t = load_w_bf(wk_d, "wk")
            wv_t = load_w_bf(wv_d, "wv")

            bk_t = []
            for d in range(CCH):
                t2 = cst.tile([128, 1], F32, name=f"bk{d}", tag=f"bk{d}")
                nc.sync.dma_start(out=t2[:], in_=bk_d[d * 128:(d + 1) * 128].unsqueeze(1))
                bk_t.append(t2)
            bv_bc = cst.tile([128, C], F32)
            nc.sync.dma_start(out=bv_bc[:], in_=bv_d[:].unsqueeze(0).broadcast_to([128, C]))

            # ---- persistent tensors ----
            kt_b = [per.tile([128, N2], BF16, name=f"ktb{cc}", tag=f"ktb{cc}")
                    for cc in range(CCH)]
            v_b = [per.tile([128, C], BF16, name=f"vb{i}", tag=f"vb{i}")
                   for i in range(KT)]

            # ---- phase X2: K^T and V ----
            for kc0 in range(N2 // 512):
                x2bt = []
                for cc in range(CCH):
                    t = sb.tile([128, 512], BF16, name=f"x2bt{cc}", tag=f"x2bt{cc}", bufs=3)
                    nc.sync.dma_start_transpose(
                        t[:], x2b_d[kc0 * 512:(kc0 + 1) * 512, cc * 128:(cc + 1) * 128])
                    x2bt.append(t)
                # K^T[d, k-block]
                for d in range(CCH):
                    pp = ps.tile([128, 512], F32, name="kps", tag="pB", bufs=3)
                    for cc in range(CCH):
                        nc.tensor.matmul(pp[:], lhsT=wk_t[cc][:, d * 128:(d + 1) * 128],
                                         rhs=x2bt[cc][:],
                                         start=(cc == 0), stop=(cc == CCH - 1))
                    nc.vector.tensor_add(
                        out=kt_b[d][:, kc0 * 512:(kc0 + 1) * 512],
                        in0=pp[:], in1=bk_t[d][:].broadcast_to([128, 512]))
                # V[k-subtile, :]
                for kb in range(4):
                    pp = ps.tile([128, C], F32, name="vps", tag="pB", bufs=3)
                    for cc in range(CCH):
                        nc.tensor.matmul(pp[:], lhsT=x2bt[cc][:, kb * 128:(kb + 1) * 128],
                                         rhs=wv_t[cc][:],
                                         start=(cc == 0), stop=(cc == CCH - 1))
                    nc.vector.tensor_add(out=v_b[kc0 * 4 + kb][:], in0=pp[:], in1=bv_bc[:])

            # ---- late weights: Q/Wqo/Wo paths (needed from chunk 0 on) ----
            wq_b = load_w_bf(wq_d, "wq")
            wqo_r = []
            for cc in range(CCH):
                stage2 = sb.tile([128, C], F32, name=f"wqos{cc}", tag="x1f1", bufs=2)
                nc.sync.dma_start(out=stage2[:], in_=wqo_d[cc * 128:(cc + 1) * 128, :])
                t2 = cst.tile([128, C], F32R, name=f"wqo{cc}", tag=f"wqo{cc}")
                nc.scalar.copy(t2[:], stage2[:])
                wqo_r.append(t2)
            wo_t = load_w_bf(wo_d, "wo")
            bq_t = []
            for d in range(CCH):
                t1 = cst.tile([128, 1], F32, name=f"bq{d}", tag=f"bq{d}")
                nc.sync.dma_start(out=t1[:], in_=bq_d[d * 128:(d + 1) * 128].unsqueeze(1))
                bq_t.append(t1)
            bo2_bc = cst.tile([128, C], F32)
            nc.sync.dma_start(out=bo2_bc[:], in_=bo2_d[:].unsqueeze(0).broadcast_to([128, C]))

            # ---- per q-chunk: transpose x1, Q^T, S^T/exp, rowsum, A^T, O ----
            for qc in range(NQC):
                q0 = qc * QC
                # x1 fp32 rows in, PE-transpose to x1t (f32r)
                x1f_in = []
                for rb in range(QC // 128):
                    t = sb.tile([128, C], F32, name=f"x1f{rb}", tag=f"x1f{rb}", bufs=2)
                    nc.sync.dma_start(out=t[:], in_=x1f_d[q0 + rb * 128:q0 + (rb + 1) * 128, :])
                    x1f_in.append(t)
                x1t_r = [sb.tile([128, QC], F32R, name=f"x1t{cc}", tag=f"x1t{cc}", bufs=2)
                         for cc in range(CCH)]
                for rb in range(QC // 128):
                    for cc in range(CCH):
                        tp = ps.tile([128, 128], F32, name="tps", tag="pA", bufs=3)
                        nc.tensor.transpose(tp[:], x1f_in[rb][:, cc * 128:(cc + 1) * 128],
                                            ident[:])
                        nc.scalar.copy(x1t_r[cc][:, rb * 128:(rb + 1) * 128], tp[:])
                # x1^T bf16 via xbar DMA for the Q projection
                x1bt = []
                for cc in range(CCH):
                    t = sb.tile([128, QC], BF16, name=f"x1bt{cc}", tag=f"x1bt{cc}", bufs=2)
                    nc.sync.dma_start_transpose(
                        t[:], x1b_d[q0:q0 + QC, cc * 128:(cc + 1) * 128])
                    x1bt.append(t)
                # Q^T (bf16) [d, q-chunk]
                qt_bf = []
                for d in range(CCH):
                    pp = ps.tile([128, QC], F32, name="qps", tag="pB", bufs=3)
                    for cc in range(CCH):
                        nc.tensor.matmul(pp[:], lhsT=wq_b[cc][:, d * 128:(d + 1) * 128],
                                         rhs=x1bt[cc][:],
                                         start=(cc == 0), stop=(cc == CCH - 1))
                    t = sb.tile([128, QC], BF16, name=f"qt{d}", tag=f"qt{d}", bufs=2)
                    nc.vector.tensor_add(out=t[:], in0=pp[:],
                                         in1=bq_t[d][:].broadcast_to([128, QC]))
                    qt_bf.append(t)
                # S^T tiles + exp -> pt[kt]
                pt = []
                for kt in range(KT):
                    pp = ps.tile([128, QC], F32, name="sps", tag="pA", bufs=3)
                    for cc in range(CCH):
                        nc.tensor.matmul(pp[:], lhsT=kt_b[cc][:, kt * 128:(kt + 1) * 128],
                                         rhs=qt_bf[cc][:],
                                         start=(cc == 0), stop=(cc == CCH - 1))
                    t = sb.tile([128, QC], BF16, name=f"pt{kt}", tag=f"pt{kt}", bufs=1)
                    nc.scalar.activation(t[:], pp[:], AF.Exp, scale=float(SCALE))
                    pt.append(t)
                # rowsum via ones-matmul over partitions, then reciprocal
                rs = ps.tile([128, QC], F32, name="rs", tag="pR", bufs=2)
                for kt in range(KT):
                    nc.tensor.matmul(rs[:], lhsT=ones_bf[:], rhs=pt[kt][:],
                                     start=(kt == 0), stop=(kt == KT - 1))
                recip = sb.tile([128, QC], F32, name="recip", tag="recip", bufs=2)
                nc.vector.reciprocal(recip[:], rs[:])
                # A^T [d, q-chunk]
                at_bf = []
                for d in range(CCH):
                    pp = ps.tile([128, QC], F32, name="aps", tag="pB", bufs=3)
                    for kt in range(KT):
                        nc.tensor.matmul(pp[:], lhsT=v_b[kt][:, d * 128:(d + 1) * 128],
                                         rhs=pt[kt][:],
                                         start=(kt == 0), stop=(kt == KT - 1))
                    t = sb.tile([128, QC], BF16, name=f"at{d}", tag=f"at{d}", bufs=2)
                    nc.vector.tensor_mul(out=t[:], in0=pp[:], in1=recip[:])
                    at_bf.append(t)
                # O = x1 @ Wqo^T (f32r) + A @ Wo^T (bf16) + bo2
                for rb in range(QC // 128):
                    pp = ps.tile([128, C], F32, name="ops", tag="pB", bufs=3)
                    for cc in range(CCH):
                        nc.tensor.matmul(pp[:], lhsT=x1t_r[cc][:, rb * 128:(rb + 1) * 128],
                                         rhs=wqo_r[cc][:],
                                         start=(cc == 0), stop=False)
                    for d in range(CCH):
                        nc.tensor.matmul(pp[:], lhsT=at_bf[d][:, rb * 128:(rb + 1) * 128],
                                         rhs=wo_t[d][:],
                                         start=False, stop=(d == CCH - 1))
                    ot = sb.tile([128, C], F32, name="ot", tag="ot", bufs=3)
                    nc.vector.tensor_add(out=ot[:], in0=pp[:], in1=bo2_bc[:])
                    nc.sync.dma_start(out=out_d[q0 + rb * 128:q0 + (rb + 1) * 128, :],
                                      in_=ot[:])

    nc.compile()
    return nc


def get_built():
    global _BUILT
    if _BUILT is None:
        _BUILT = build()
    return _BUILT


def make_in_maps(x1, x2, Wq, bq, Wk, bk, Wv, bv, Wo, bo):
    bf = ml_dtypes.bfloat16
    wq_t = np.ascontiguousarray(Wq.T).astype(bf)
    wk_t = np.ascontiguousarray(Wk.T).astype(bf)
    wv_t = np.ascontiguousarray(Wv.T).astype(bf)
    wo_t = np.ascontiguousarray(Wo.T).astype(bf)
    wqo_t = np.ascontiguousarray((Wo @ Wq).T).astype(np.float32)
    bo2 = (Wo @ bq + bo).astype(np.float32)
    in_maps = []
    for cid in range(NCORES):
        b, h = cid // 2, cid % 2
        x1s = np.ascontiguousarray(x1[b, h * QROWS:(h + 1) * QROWS, :])
        in_maps.append({
            "x1f": x1s,
            "x1b": x1s.astype(bf),
            "x2b": np.ascontiguousarray(x2[b]).astype(bf),
            "wq_t": wq_t, "wk_t": wk_t, "wv_t": wv_t, "wo_t": wo_t,
            "wqo_t": wqo_t,
            "bq": bq.astype(np.float32), "bk": bk.astype(np.float32),
            "bv": bv.astype(np.float32), "bo2": bo2,
        })
    return in_maps


LAST_RESULT = None


def kernel(x1, x2, Wq, bq, Wk, bk, Wv, bv, Wo, bo):
    global LAST_RESULT
    nc = get_built()
    in_maps = make_in_maps(x1, x2, Wq, bq, Wk, bk, Wv, bv, Wo, bo)
    trace = bool(os.environ.get("KERNEL_TRACE"))
    res = run_bass_kernel_spmd(nc, in_maps, core_ids=list(range(NCORES)), trace=trace)
    LAST_RESULT = res
    out = np.empty((B, N1, C), dtype=np.float32)
    for cid in range(NCORES):
        b, h = cid // 2, cid % 2
        out[b, h * QROWS:(h + 1) * QROWS, :] = res.results[cid]["out"]
    return out



# revision 5
# speedup vs baseline: 2.2732x; 2.2732x over previous
"""CrossFeatureAttention TRN2 kernel (fp8 DoubleRow attention).

Full inputs -> full output. Sharding: data-parallel over (batch b, half of N1)
across 8 cores; each core computes out[b, h*2048:(h+1)*2048, :].

Math (per core, x1 slice q=2048 rows, x2[b] k=4096 rows, C=512):
    Q  = x1 @ Wq^T + bq              (bf16 matmul, fp32 kept for the residual)
    K^T= Wk x2^T + bk                (fp8 DoubleRow)
    V  = x2 @ Wv^T + bv              (fp8 DoubleRow)
    P  = exp(Q K^T / sqrt(C))        (fp8 DoubleRow scores, ACT exp -> fp8)
    rs = colsum(P^T)                 (ones-matmul, fp8 DoubleRow)
    U^T= Q^T + (V^T P^T) * recip(rs) (PV fp8 DoubleRow, fp32 combine on DVE)
    out= U @ Wo^T + bo               (f32r matmul at full rate)

All fp8 matmuls use MatmulPerfMode.DoubleRow: operands hold contraction
k-tile PAIRS in a [128, 2, F] layout (dim1 = k-tile index), halving PE
cycles vs bf16.  The residual path stays fp32: Q^T is kept from PSUM and
added to the normalized attention output on DVE, and the output projection
runs with float32r operands (1 cycle/row at free-dim 512).

Chunks of 512 q-rows are software-pipelined: chunk i+1's Q-projection and
S-matmuls are issued before chunk i's rowsum/PV/out so the ACT exp tail of
chunk i hides behind PE work.
"""

import os
import sys

import numpy as np

for _p in ("/root/.axon_site", "/root/.axon_site/_ro/trn_rl_repo",
           "/root/.axon_site/_ro/pypackages"):
    if _p not in sys.path and os.path.isdir(_p):
        sys.path.append(_p)

import ml_dtypes

import concourse.bacc as bacc
import concourse.mybir as mybir
import concourse.tile as tile
from concourse.bass_utils import run_bass_kernel_spmd

F32 = mybir.dt.float32
F32R = mybir.dt.float32r
BF16 = mybir.dt.bfloat16
F8 = mybir.dt.float8e4
AF = mybir.ActivationFunctionType
DR = mybir.MatmulPerfMode.DoubleRow

B, N1, N2, C = 4, 4096, 4096, 512
NCORES = 8
QROWS = N1 * B // NCORES          # 2048 q rows per core
QC = 512                          # q-chunk
NQC = QROWS // QC                 # 4 chunks
KT = N2 // 128                    # 32 k-tiles
CCH = C // 128                    # 4 contraction chunks
SCALE = 1.0 / float(np.sqrt(C))

_BUILT = None


def build():
    nc = bacc.Bacc(None, target_bir_lowering=False, debug=False)

    x1t_d = nc.dram_tensor("x1t", [C, QROWS], BF16, kind="ExternalInput")
    x2t_d = nc.dram_tensor("x2t", [128, CCH, N2], F8, kind="ExternalInput")
    wq_d = nc.dram_tensor("wq_t", [C, C], BF16, kind="ExternalInput")
    wk_d = nc.dram_tensor("wk8", [128, CCH, C], F8, kind="ExternalInput")
    wv_d = nc.dram_tensor("wv8", [128, CCH, C], F8, kind="ExternalInput")
    wo_d = nc.dram_tensor("wo_t", [C, C], F32, kind="ExternalInput")
    bq_d = nc.dram_tensor("bq", [C], F32, kind="ExternalInput")
    bk_d = nc.dram_tensor("bk", [C], F32, kind="ExternalInput")
    bv_d = nc.dram_tensor("bv", [C], F32, kind="ExternalInput")
    bo_d = nc.dram_tensor("bo", [C], F32, kind="ExternalInput")
    out_d = nc.dram_tensor("out", [QROWS, C], F32, kind="ExternalOutput")

    with tile.TileContext(nc) as tc:
        with tc.tile_pool(name="cst", bufs=1) as cst, \
             tc.tile_pool(name="per", bufs=1) as per, \
             tc.tile_pool(name="sb", bufs=1) as sb, \
             tc.tile_pool(name="ps", bufs=1, space="PSUM") as ps:

            # ---- constants / weights ----
            ones2 = cst.tile([128, 2, 128], F8)
            nc.gpsimd.memset(ones2[:], 1.0)

            wq_b = []
            for cc in range(CCH):
                t = cst.tile([128, C], BF16, name=f"wq{cc}", tag=f"wq{cc}")
                nc.sync.dma_start(out=t[:], in_=wq_d[cc * 128:(cc + 1) * 128, :])
                wq_b.append(t)
            bq_t = []
            for d in range(CCH):
                t1 = cst.tile([128, 1], F32, name=f"bq{d}", tag=f"bq{d}")
                nc.sync.dma_start(out=t1[:], in_=bq_d[d * 128:(d + 1) * 128].unsqueeze(1))
                bq_t.append(t1)
            x1t = []
            for cc in range(CCH):
                t = cst.tile([128, QROWS], BF16, name=f"x1t{cc}", tag=f"x1t{cc}")
                nc.sync.dma_start(out=t[:], in_=x1t_d[cc * 128:(cc + 1) * 128, :])
                x1t.append(t)

            wk8 = cst.tile([128, CCH, C], F8, name="wk8", tag="wk8")
            nc.sync.dma_start(out=wk8[:], in_=wk_d[:])
            wv8 = cst.tile([128, CCH, C], F8, name="wv8", tag="wv8")
            nc.sync.dma_start(out=wv8[:], in_=wv_d[:])
            bk_t = []
            for d in range(CCH):
                t2 = cst.tile([128, 1], F32, name=f"bk{d}", tag=f"bk{d}")
                nc.sync.dma_start(out=t2[:], in_=bk_d[d * 128:(d + 1) * 128].unsqueeze(1))
                bk_t.append(t2)
            bv_bc = cst.tile([128, C], F32)
            nc.sync.dma_start(out=bv_bc[:], in_=bv_d[:].unsqueeze(0).broadcast_to([128, C]))

            x2t = per.tile([128, CCH, N2], F8, name="x2t", tag="x2t")
            nc.sync.dma_start(out=x2t[:], in_=x2t_d[:])

            wo_t = []
            for d in range(CCH):
                stage = sb.tile([128, C], F32, name=f"wos{d}", tag="wos", bufs=2)
                nc.sync.dma_start(out=stage[:], in_=wo_d[d * 128:(d + 1) * 128, :])
                t = cst.tile([128, C], F32R, name=f"wo{d}", tag=f"wo{d}")
                nc.scalar.copy(t[:], stage[:])
                wo_t.append(t)
            bo_bc = cst.tile([128, C], F32)
            nc.sync.dma_start(out=bo_bc[:], in_=bo_d[:].unsqueeze(0).broadcast_to([128, C]))

            # ---- persistent attention operands ----
            kt8 = per.tile([128, CCH, N2], F8, name="kt8", tag="kt8")
            v8 = [per.tile([128, 2, C], F8, name=f"v8_{j}", tag=f"v8_{j}")
                  for j in range(KT // 2)]

            # ---- per-chunk state (double buffered across chunks) ----
            def chunk_tiles():
                return {
                    "qtf": [sb.tile([128, QC], F32, name=f"qtf{d}", tag=f"qtf{d}",
                                    bufs=2) for d in range(CCH)],
                    "qt8": sb.tile([128, CCH, QC], F8, name="qt8", tag="qt8", bufs=2),
                    "pt8": [sb.tile([128, 2, QC], F8, name=f"pt{j}", tag=f"pt{j}",
                                    bufs=2) for j in range(KT // 2)],
                    "recip": sb.tile([128, QC], F32, name="recip", tag="recip", bufs=2),
                    "u": [sb.tile([128, QC], F32R, name=f"u{d}", tag=f"u{d}", bufs=2)
                          for d in range(CCH)],
                }

            st = [None] * NQC

            def emit_q(i):
                st[i] = chunk_tiles()
                q0 = i * QC
                for d in range(CCH):
                    qp = ps.tile([128, QC], F32, name="qp", tag="pB", bufs=4)
                    for cc in range(CCH):
                        nc.tensor.matmul(qp[:], lhsT=wq_b[cc][:, d * 128:(d + 1) * 128],
                                         rhs=x1t[cc][:, q0:q0 + QC],
                                         start=(cc == 0), stop=(cc == CCH - 1))
                    nc.vector.tensor_add(out=st[i]["qtf"][d][:], in0=qp[:],
                                         in1=bq_t[d][:].broadcast_to([128, QC]))
                    nc.scalar.activation(st[i]["qt8"][:, d, :], qp[:], AF.Identity,
                                         bias=bq_t[d][:])

            def emit_x2_phase():
                for kc0 in range(N2 // 512):
                    for d in range(CCH):
                        pp = ps.tile([128, 512], F32, name="kps", tag="pB", bufs=4)
                        for j2 in range(2):
                            nc.tensor.matmul(
                                pp[:],
                                lhsT=wk8[:, 2 * j2:2 * j2 + 2, d * 128:(d + 1) * 128],
                                rhs=x2t[:, 2 * j2:2 * j2 + 2, kc0 * 512:(kc0 + 1) * 512],
                                start=(j2 == 0), stop=(j2 == 1), perf_mode=DR)
                        nc.vector.tensor_add(
                            out=kt8[:, d, kc0 * 512:(kc0 + 1) * 512],
                            in0=pp[:], in1=bk_t[d][:].broadcast_to([128, 512]))
                    for kb in range(4):
                        kt = kc0 * 4 + kb
                        pp = ps.tile([128, C], F32, name="vps", tag="pB", bufs=4)
                        for j2 in range(2):
                            nc.tensor.matmul(
                                pp[:],
                                lhsT=x2t[:, 2 * j2:2 * j2 + 2, kt * 128:(kt + 1) * 128],
                                rhs=wv8[:, 2 * j2:2 * j2 + 2, :],
                                start=(j2 == 0), stop=(j2 == 1), perf_mode=DR)
                        nc.vector.tensor_add(out=v8[kt // 2][:, kt % 2, :],
                                             in0=pp[:], in1=bv_bc[:])

            def emit_s(i):
                for kt in range(KT):
                    sp = ps.tile([128, QC], F32, name="sps", tag="pA", bufs=3)
                    for j2 in range(2):
                        nc.tensor.matmul(
                            sp[:],
                            lhsT=kt8[:, 2 * j2:2 * j2 + 2, kt * 128:(kt + 1) * 128],
                            rhs=st[i]["qt8"][:, 2 * j2:2 * j2 + 2, :],
                            start=(j2 == 0), stop=(j2 == 1), perf_mode=DR)
                    nc.scalar.activation(st[i]["pt8"][kt // 2][:, kt % 2, :], sp[:],
                                         AF.Exp, scale=float(SCALE))

            def emit_r(i):
                rs = ps.tile([128, QC], F32, name="rs", tag="pR", bufs=1)
                for j in range(KT // 2):
                    nc.tensor.matmul(rs[:], lhsT=ones2[:], rhs=st[i]["pt8"][j][:],
                                     start=(j == 0), stop=(j == KT // 2 - 1),
                                     perf_mode=DR)
                nc.vector.reciprocal(st[i]["recip"][:], rs[:])

            def emit_pv_u(i):
                for d in range(CCH):
                    ap = ps.tile([128, QC], F32, name="aps", tag="pB", bufs=4)
                    for j in range(KT // 2):
                        nc.tensor.matmul(ap[:],
                                         lhsT=v8[j][:, :, d * 128:(d + 1) * 128],
                                         rhs=st[i]["pt8"][j][:],
                                         start=(j == 0), stop=(j == KT // 2 - 1),
                                         perf_mode=DR)
                    at = sb.tile([128, QC], F32, name="at", tag="at", bufs=2)
                    nc.vector.tensor_mul(out=at[:], in0=ap[:], in1=st[i]["recip"][:])
                    nc.vector.tensor_add(out=st[i]["u"][d][:], in0=at[:],
                                         in1=st[i]["qtf"][d][:])

            def emit_o(i):
                q0 = i * QC
                for rb in range(QC // 128):
                    op = ps.tile([128, C], F32, name="ops", tag="pB", bufs=4)
                    for d in range(CCH):
                        nc.tensor.matmul(
                            op[:],
                            lhsT=st[i]["u"][d][:, rb * 128:(rb + 1) * 128],
                            rhs=wo_t[d][:],
                            start=(d == 0), stop=(d == CCH - 1))
                    ot = sb.tile([128, C], F32, name="ot", tag="ot", bufs=3)
                    nc.vector.tensor_add(out=ot[:], in0=op[:], in1=bo_bc[:])
                    nc.sync.dma_start(out=out_d[q0 + rb * 128:q0 + (rb + 1) * 128, :],
                                      in_=ot[:])

            emit_q(0)
            emit_x2_phase()
            emit_s(0)
            for i in range(1, NQC):
                emit_q(i)
                emit_s(i)
                emit_r(i - 1)
                emit_pv_u(i - 1)
                emit_o(i - 1)
            emit_r(NQC - 1)
            emit_pv_u(NQC - 1)
            emit_o(NQC - 1)

    nc.compile()
    return nc


def get_built():
    global _BUILT
    if _BUILT is None:
        _BUILT = build()
    return _BUILT


def _pair_layout(a, f8):
    # [512 (contract), X] -> [128, 4, X] fp8: [p, j, x] = a[j*128+p, x]
    return np.ascontiguousarray(a.reshape(CCH, 128, -1).transpose(1, 0, 2)).astype(f8)


def make_in_maps(x1, x2, Wq, bq, Wk, bk, Wv, bv, Wo, bo):
    bf = ml_dtypes.bfloat16
    f8 = ml_dtypes.float8_e4m3
    wq_t = np.ascontiguousarray(Wq.T).astype(bf)
    wk8 = _pair_layout(np.ascontiguousarray(Wk.T), f8)
    wv8 = _pair_layout(np.ascontiguousarray(Wv.T), f8)
    wo_t = np.ascontiguousarray(Wo.T).astype(np.float32)
    bq32 = bq.astype(np.float32)
    bk32 = bk.astype(np.float32)
    bv32 = bv.astype(np.float32)
    bo32 = bo.astype(np.float32)
    x2t8 = [_pair_layout(np.ascontiguousarray(x2[b].T), f8) for b in range(B)]
    in_maps = []
    for cid in range(NCORES):
        b, h = cid // 2, cid % 2
        x1s = x1[b, h * QROWS:(h + 1) * QROWS, :]
        in_maps.append({
            "x1t": np.ascontiguousarray(x1s.T).astype(bf),
            "x2t": x2t8[b],
            "wq_t": wq_t, "wk8": wk8, "wv8": wv8, "wo_t": wo_t,
            "bq": bq32, "bk": bk32, "bv": bv32, "bo": bo32,
        })
    return in_maps


LAST_RESULT = None


def kernel(x1, x2, Wq, bq, Wk, bk, Wv, bv, Wo, bo):
    global LAST_RESULT
    nc = get_built()
    in_maps = make_in_maps(x1, x2, Wq, bq, Wk, bk, Wv, bv, Wo, bo)
    trace = bool(os.environ.get("KERNEL_TRACE"))
    res = run_bass_kernel_spmd(nc, in_maps, core_ids=list(range(NCORES)), trace=trace)
    LAST_RESULT = res
    out = np.empty((B, N1, C), dtype=np.float32)
    for cid in range(NCORES):
        b, h = cid // 2, cid % 2
        out[b, h * QROWS:(h + 1) * QROWS, :] = res.results[cid]["out"]
    return out


# revision 15
# speedup vs baseline: 2.3148x; 1.0183x over previous
"""CrossFeatureAttention TRN2 kernel (fp8 DoubleRow attention).

Full inputs -> full output. Sharding: data-parallel over (batch b, half of N1)
across 8 cores; each core computes out[b, h*2048:(h+1)*2048, :].

Math (per core, x1 slice q=2048 rows, x2[b] k=4096 rows, C=512):
    Q  = x1 @ Wq^T + bq              (bf16 matmul, fp32 kept for the residual)
    K^T= Wk x2^T + bk                (fp8 DoubleRow)
    V  = x2 @ Wv^T + bv              (fp8 DoubleRow)
    P  = exp(Q K^T / sqrt(C))        (fp8 DoubleRow scores, ACT exp -> fp8)
    rs = colsum(P^T)                 (ones-matmul, fp8 DoubleRow)
    U^T= Q^T + (V^T P^T) * recip(rs) (PV fp8 DoubleRow, fp32 combine on DVE)
    out= U @ Wo^T + bo               (f32r matmul at full rate)

All fp8 matmuls use MatmulPerfMode.DoubleRow: operands hold contraction
k-tile PAIRS in a [128, 2, F] layout (dim1 = k-tile index), halving PE
cycles vs bf16.  The residual path stays fp32: Q^T is kept from PSUM and
added to the normalized attention output on DVE, and the output projection
runs with float32r operands (1 cycle/row at free-dim 512).

Chunks of 512 q-rows are software-pipelined: chunk i+1's Q-projection and
S-matmuls are issued before chunk i's rowsum/PV/out so the ACT exp tail of
chunk i hides behind PE work.
"""

import os
import sys

import numpy as np

for _p in ("/root/.axon_site", "/root/.axon_site/_ro/trn_rl_repo",
           "/root/.axon_site/_ro/pypackages"):
    if _p not in sys.path and os.path.isdir(_p):
        sys.path.append(_p)

import ml_dtypes

import concourse.bacc as bacc
import concourse.mybir as mybir
import concourse.tile as tile
from concourse.bass_utils import run_bass_kernel_spmd

F32 = mybir.dt.float32
F32R = mybir.dt.float32r
BF16 = mybir.dt.bfloat16
F8 = mybir.dt.float8e4
AF = mybir.ActivationFunctionType
DR = mybir.MatmulPerfMode.DoubleRow

B, N1, N2, C = 4, 4096, 4096, 512
NCORES = 8
QROWS = N1 * B // NCORES          # 2048 q rows per core
QC = 512                          # q-chunk
NQC = QROWS // QC                 # 4 chunks
KT = N2 // 128                    # 32 k-tiles
CCH = C // 128                    # 4 contraction chunks
SCALE = 1.0 / float(np.sqrt(C))

_BUILT = None


def build():
    nc = bacc.Bacc(None, target_bir_lowering=False, debug=False)

    x1t_d = nc.dram_tensor("x1t", [C, QROWS], BF16, kind="ExternalInput")
    x2t_d = nc.dram_tensor("x2t", [128, CCH * N2], F8, kind="ExternalInput")
    wq_d = nc.dram_tensor("wq_t", [C, C], BF16, kind="ExternalInput")
    wk_d = nc.dram_tensor("wk8", [128, CCH, C], F8, kind="ExternalInput")
    wv_d = nc.dram_tensor("wv8", [128, CCH, C], F8, kind="ExternalInput")
    wo_d = nc.dram_tensor("wo_t", [C, C], F32, kind="ExternalInput")
    bq_d = nc.dram_tensor("bq", [C], F32, kind="ExternalInput")
    bk_d = nc.dram_tensor("bk", [C], F32, kind="ExternalInput")
    bv_d = nc.dram_tensor("bv", [C], F32, kind="ExternalInput")
    bo_d = nc.dram_tensor("bo", [C], F32, kind="ExternalInput")
    out_d = nc.dram_tensor("out", [QROWS, C], F32, kind="ExternalOutput")

    with tile.TileContext(nc) as tc:
        with tc.tile_pool(name="cst", bufs=1) as cst, \
             tc.tile_pool(name="per", bufs=1) as per, \
             tc.tile_pool(name="sb", bufs=1) as sb, \
             tc.tile_pool(name="ps", bufs=1, space="PSUM") as ps:

            # ---- constants / weights ----
            ones2 = cst.tile([128, 2, 128], F8)
            nc.gpsimd.memset(ones2[:], 1.0)

            x1t = []
            for cc in range(CCH):
                t = cst.tile([128, QROWS], BF16, name=f"x1t{cc}", tag=f"x1t{cc}")
                nc.sync.dma_start(out=t[:], in_=x1t_d[cc * 128:(cc + 1) * 128, :])
                x1t.append(t)
            wq_b = []
            for cc in range(CCH):
                t = cst.tile([128, C], BF16, name=f"wq{cc}", tag=f"wq{cc}")
                nc.sync.dma_start(out=t[:], in_=wq_d[cc * 128:(cc + 1) * 128, :])
                wq_b.append(t)
            bq_t = []
            for d in range(CCH):
                t1 = cst.tile([128, 1], F32, name=f"bq{d}", tag=f"bq{d}")
                nc.sync.dma_start(out=t1[:], in_=bq_d[d * 128:(d + 1) * 128].unsqueeze(1))
                bq_t.append(t1)

            wk8 = cst.tile([128, CCH, C], F8, name="wk8", tag="wk8")
            nc.sync.dma_start(out=wk8[:], in_=wk_d[:])
            wv8 = cst.tile([128, CCH, C], F8, name="wv8", tag="wv8")
            nc.sync.dma_start(out=wv8[:], in_=wv_d[:])
            bk_t = []
            for d in range(CCH):
                t2 = cst.tile([128, 1], F32, name=f"bk{d}", tag=f"bk{d}")
                nc.sync.dma_start(out=t2[:], in_=bk_d[d * 128:(d + 1) * 128].unsqueeze(1))
                bk_t.append(t2)
            bv_bc = cst.tile([128, C], F32)
            nc.sync.dma_start(out=bv_bc[:], in_=bv_d[:].unsqueeze(0).broadcast_to([128, C]))

            # x2^T in 8 k-block pieces so the X2 phase can start on block 0
            x2t = per.tile([128, CCH, N2], F8, name="x2t", tag="x2t")
            for kc0 in range(N2 // 512):
                for j in range(CCH):
                    nc.sync.dma_start(
                        out=x2t[:, j, kc0 * 512:(kc0 + 1) * 512],
                        in_=x2t_d[:, j * N2 + kc0 * 512:j * N2 + (kc0 + 1) * 512])

            wo_t = []
            for d in range(CCH):
                stage = sb.tile([128, C], F32, name=f"wos{d}", tag="wos", bufs=2)
                nc.sync.dma_start(out=stage[:], in_=wo_d[d * 128:(d + 1) * 128, :])
                t = cst.tile([128, C], F32R, name=f"wo{d}", tag=f"wo{d}")
                nc.scalar.copy(t[:], stage[:])
                wo_t.append(t)
            bo_bc = cst.tile([128, C], F32)
            nc.sync.dma_start(out=bo_bc[:], in_=bo_d[:].unsqueeze(0).broadcast_to([128, C]))

            # ---- persistent attention operands ----
            kt8 = per.tile([128, CCH, N2], F8, name="kt8", tag="kt8")
            v8 = [per.tile([128, 2, C], F8, name=f"v8_{j}", tag=f"v8_{j}")
                  for j in range(KT // 2)]

            # ---- per-chunk state (double buffered across chunks) ----
            def chunk_tiles():
                return {
                    "qtf": [sb.tile([128, QC], F32, name=f"qtf{d}", tag=f"qtf{d}",
                                    bufs=2) for d in range(CCH)],
                    "qt8": sb.tile([128, CCH, QC], F8, name="qt8", tag="qt8", bufs=2),
                    "pt8": [sb.tile([128, 2, QC], F8, name=f"pt{j}", tag=f"pt{j}",
                                    bufs=2) for j in range(KT // 2)],
                    "recip": sb.tile([128, QC], F32, name="recip", tag="recip", bufs=2),
                    "u": [sb.tile([128, QC], F32R, name=f"u{d}", tag=f"u{d}", bufs=2)
                          for d in range(CCH)],
                }

            st = [None] * NQC

            def emit_q(i):
                st[i] = chunk_tiles()
                q0 = i * QC
                for d in range(CCH):
                    qp = ps.tile([128, QC], F32, name="qp", tag="pB", bufs=4)
                    for cc in range(CCH):
                        nc.tensor.matmul(qp[:], lhsT=wq_b[cc][:, d * 128:(d + 1) * 128],
                                         rhs=x1t[cc][:, q0:q0 + QC],
                                         start=(cc == 0), stop=(cc == CCH - 1))
                    nc.vector.tensor_add(out=st[i]["qtf"][d][:], in0=qp[:],
                                         in1=bq_t[d][:].broadcast_to([128, QC]))
                    nc.scalar.activation(st[i]["qt8"][:, d, :], qp[:], AF.Identity,
                                         bias=bq_t[d][:])

            def emit_x2_group(kc0):
                # K^T d-chunks: casts split DVE (d0,d1) / ACT (d2,d3)
                for d in range(CCH):
                    pp = ps.tile([128, 512], F32, name="kps", tag="pB", bufs=4)
                    for j2 in range(2):
                        nc.tensor.matmul(
                            pp[:],
                            lhsT=wk8[:, 2 * j2:2 * j2 + 2, d * 128:(d + 1) * 128],
                            rhs=x2t[:, 2 * j2:2 * j2 + 2, kc0 * 512:(kc0 + 1) * 512],
                            start=(j2 == 0), stop=(j2 == 1), perf_mode=DR)
                    if d < 2:
                        nc.vector.tensor_add(
                            out=kt8[:, d, kc0 * 512:(kc0 + 1) * 512], in0=pp[:],
                            in1=bk_t[d][:].broadcast_to([128, 512]))
                    else:
                        nc.scalar.activation(kt8[:, d, kc0 * 512:(kc0 + 1) * 512],
                                             pp[:], AF.Identity, bias=bk_t[d][:])
                for kb in range(4):
                    kt = kc0 * 4 + kb
                    pp = ps.tile([128, C], F32, name="vps", tag="pB", bufs=4)
                    for j2 in range(2):
                        nc.tensor.matmul(
                            pp[:],
                            lhsT=x2t[:, 2 * j2:2 * j2 + 2, kt * 128:(kt + 1) * 128],
                            rhs=wv8[:, 2 * j2:2 * j2 + 2, :],
                            start=(j2 == 0), stop=(j2 == 1), perf_mode=DR)
                    nc.vector.tensor_add(out=v8[kt // 2][:, kt % 2, :],
                                         in0=pp[:], in1=bv_bc[:])

            def emit_s_kt(i, kt):
                sp = ps.tile([128, QC], F32, name="sps", tag="pA", bufs=3)
                for j2 in range(2):
                    nc.tensor.matmul(
                        sp[:],
                        lhsT=kt8[:, 2 * j2:2 * j2 + 2, kt * 128:(kt + 1) * 128],
                        rhs=st[i]["qt8"][:, 2 * j2:2 * j2 + 2, :],
                        start=(j2 == 0), stop=(j2 == 1), perf_mode=DR)
                nc.scalar.activation(st[i]["pt8"][kt // 2][:, kt % 2, :], sp[:],
                                     AF.Exp, scale=float(SCALE))

            def emit_r_j(i, j, rs):
                nc.tensor.matmul(rs[:], lhsT=ones2[:], rhs=st[i]["pt8"][j][:],
                                 start=(j == 0), stop=(j == KT // 2 - 1),
                                 perf_mode=DR)

            def emit_pv_j(i, j, ap):
                for d in range(CCH):
                    nc.tensor.matmul(ap[d][:],
                                     lhsT=v8[j][:, :, d * 128:(d + 1) * 128],
                                     rhs=st[i]["pt8"][j][:],
                                     start=(j == 0), stop=(j == KT // 2 - 1),
                                     perf_mode=DR)

            def emit_recip(i, rs):
                nc.vector.reciprocal_approx_fast(out=st[i]["recip"][:], in_=rs[:])

            def emit_u(i, ap):
                for d in range(CCH):
                    at = sb.tile([128, QC], F32, name="at", tag="at", bufs=2)
                    nc.vector.tensor_mul(out=at[:], in0=ap[d][:],
                                         in1=st[i]["recip"][:])
                    nc.vector.tensor_add(out=st[i]["u"][d][:], in0=at[:],
                                         in1=st[i]["qtf"][d][:])

            def emit_o(i):
                q0 = i * QC
                for rb in range(QC // 128):
                    op = ps.tile([128, C], F32, name="ops", tag="pB", bufs=4)
                    for d in range(CCH):
                        nc.tensor.matmul(
                            op[:],
                            lhsT=st[i]["u"][d][:, rb * 128:(rb + 1) * 128],
                            rhs=wo_t[d][:],
                            start=(d == 0), stop=(d == CCH - 1))
                    ot = sb.tile([128, C], F32, name="ot", tag="ot", bufs=3)
                    nc.vector.tensor_add(out=ot[:], in0=op[:], in1=bo_bc[:])
                    nc.sync.dma_start(out=out_d[q0 + rb * 128:q0 + (rb + 1) * 128, :],
                                      in_=ot[:])

            # ---- schedule ----
            emit_q(0)
            # X2 phase with chunk-0 S matmuls interleaved once their
            # K-blocks are ready
            s0_next = 0
            for kc0 in range(N2 // 512):
                emit_x2_group(kc0)
                if kc0 >= 2:
                    target = min(3 * (kc0 - 1), 4 * kc0 + 4)
                    while s0_next < target:
                        emit_s_kt(0, s0_next)
                        s0_next += 1
            while s0_next < KT:
                emit_s_kt(0, s0_next)
                s0_next += 1
            emit_q(1)

            # steady state: chunk i's S interleaved with chunk i-1's
            # rowsum/PV groups (whose inputs are long since ready)
            for i in range(1, NQC):
                rs = ps.tile([128, QC], F32, name="rs", tag="pR", bufs=1)
                ap = [ps.tile([128, QC], F32, name="aps", tag="pB", bufs=4)
                      for _ in range(CCH)]
                for j in range(KT // 2):
                    emit_s_kt(i, 2 * j)
                    emit_s_kt(i, 2 * j + 1)
                    emit_r_j(i - 1, j, rs)
                    emit_pv_j(i - 1, j, ap)
                emit_recip(i - 1, rs)
                emit_u(i - 1, ap)
                if i + 1 < NQC:
                    emit_q(i + 1)
                emit_o(i - 1)

            # last chunk: batched rowsum (exp done by now), PV per-d with
            # immediate normalize+residual so O starts early
            i = NQC - 1
            rs = ps.tile([128, QC], F32, name="rs", tag="pR", bufs=1)
            for j in range(KT // 2):
                emit_r_j(i, j, rs)
            emit_recip(i, rs)
            for d in range(CCH):
                apd = ps.tile([128, QC], F32, name="aps", tag="pB", bufs=4)
                for j in range(KT // 2):
                    nc.tensor.matmul(apd[:],
                                     lhsT=v8[j][:, :, d * 128:(d + 1) * 128],
                                     rhs=st[i]["pt8"][j][:],
                                     start=(j == 0), stop=(j == KT // 2 - 1),
                                     perf_mode=DR)
                at = sb.tile([128, QC], F32, name="at", tag="at", bufs=2)
                nc.vector.tensor_mul(out=at[:], in0=apd[:], in1=st[i]["recip"][:])
                nc.vector.tensor_add(out=st[i]["u"][d][:], in0=at[:],
                                     in1=st[i]["qtf"][d][:])
            emit_o(i)

    nc.compile()
    return nc


def get_built():
    global _BUILT
    if _BUILT is None:
        _BUILT = build()
    return _BUILT


def _pair_layout(a, f8):
    # [512 (contract), X] -> [128, 4, X] fp8: [p, j, x] = a[j*128+p, x]
    return np.ascontiguousarray(a.reshape(CCH, 128, -1).transpose(1, 0, 2)).astype(f8)


def make_in_maps(x1, x2, Wq, bq, Wk, bk, Wv, bv, Wo, bo):
    bf = ml_dtypes.bfloat16
    f8 = ml_dtypes.float8_e4m3
    wq_t = np.ascontiguousarray(Wq.T).astype(bf)
    wk8 = _pair_layout(np.ascontiguousarray(Wk.T), f8)
    wv8 = _pair_layout(np.ascontiguousarray(Wv.T), f8)
    wo_t = np.ascontiguousarray(Wo.T).astype(np.float32)
    bq32 = bq.astype(np.float32)
    bk32 = bk.astype(np.float32)
    bv32 = bv.astype(np.float32)
    bo32 = bo.astype(np.float32)
    x2t8 = [_pair_layout(np.ascontiguousarray(x2[b].T), f8).reshape(128, -1)
            for b in range(B)]
    in_maps = []
    for cid in range(NCORES):
        b, h = cid // 2, cid % 2
        x1s = x1[b, h * QROWS:(h + 1) * QROWS, :]
        in_maps.append({
            "x1t": np.ascontiguousarray(x1s.T).astype(bf),
            "x2t": x2t8[b],
            "wq_t": wq_t, "wk8": wk8, "wv8": wv8, "wo_t": wo_t,
            "bq": bq32, "bk": bk32, "bv": bv32, "bo": bo32,
        })
    return in_maps


LAST_RESULT = None


def kernel(x1, x2, Wq, bq, Wk, bk, Wv, bv, Wo, bo):
    global LAST_RESULT
    nc = get_built()
    in_maps = make_in_maps(x1, x2, Wq, bq, Wk, bk, Wv, bv, Wo, bo)
    trace = bool(os.environ.get("KERNEL_TRACE"))
    res = run_bass_kernel_spmd(nc, in_maps, core_ids=list(range(NCORES)), trace=trace)
    LAST_RESULT = res
    out = np.empty((B, N1, C), dtype=np.float32)
    for cid in range(NCORES):
        b, h = cid // 2, cid % 2
        out[b, h * QROWS:(h + 1) * QROWS, :] = res.results[cid]["out"]
    return out


# revision 24
# speedup vs baseline: 2.3586x; 1.0189x over previous
"""CrossFeatureAttention TRN2 kernel (fp8 DoubleRow attention).

Full inputs -> full output. Sharding: data-parallel over (batch b, half of N1)
across 8 cores; each core computes out[b, h*2048:(h+1)*2048, :].

Math (per core, x1 slice q=2048 rows, x2[b] k=4096 rows, C=512):
    Q  = x1 @ Wq^T + bq              (bf16 matmul, fp32 kept for the residual)
    K^T= Wk x2^T + bk                (fp8 DoubleRow)
    V  = x2 @ Wv^T + bv              (fp8 DoubleRow)
    P  = exp(Q K^T / sqrt(C))        (fp8 DoubleRow scores, ACT exp -> fp8)
    rs = colsum(P^T)                 (ones-matmul, fp8 DoubleRow)
    U^T= Q^T + (V^T P^T) * recip(rs) (PV fp8 DoubleRow, fp32 combine on DVE)
    out= U @ Wo^T + bo               (f32r matmul at full rate)

All fp8 matmuls use MatmulPerfMode.DoubleRow: operands hold contraction
k-tile PAIRS in a [128, 2, F] layout (dim1 = k-tile index), halving PE
cycles vs bf16.  The residual path stays fp32: Q^T is kept from PSUM and
added to the normalized attention output on DVE, and the output projection
runs with float32r operands (1 cycle/row at free-dim 512).

Chunks of 512 q-rows are software-pipelined: chunk i+1's Q-projection and
S-matmuls are issued before chunk i's rowsum/PV/out so the ACT exp tail of
chunk i hides behind PE work.
"""

import os
import sys

import numpy as np

for _p in ("/root/.axon_site", "/root/.axon_site/_ro/trn_rl_repo",
           "/root/.axon_site/_ro/pypackages"):
    if _p not in sys.path and os.path.isdir(_p):
        sys.path.append(_p)

import ml_dtypes

import concourse.bacc as bacc
import concourse.mybir as mybir
import concourse.tile as tile
from concourse.bass_utils import run_bass_kernel_spmd

F32 = mybir.dt.float32
F32R = mybir.dt.float32r
BF16 = mybir.dt.bfloat16
F8 = mybir.dt.float8e4
AF = mybir.ActivationFunctionType
DR = mybir.MatmulPerfMode.DoubleRow

B, N1, N2, C = 4, 4096, 4096, 512
NCORES = 8
QROWS = N1 * B // NCORES          # 2048 q rows per core
QC = 512                          # q-chunk
NQC = QROWS // QC                 # 4 chunks
KT = N2 // 128                    # 32 k-tiles
CCH = C // 128                    # 4 contraction chunks
SCALE = 1.0 / float(np.sqrt(C))

_BUILT = None


def build():
    nc = bacc.Bacc(None, target_bir_lowering=False, debug=False)

    x1t_d = nc.dram_tensor("x1t", [128, CCH, QROWS], BF16, kind="ExternalInput")
    x2t_d = nc.dram_tensor("x2t", [128, CCH, N2], F8, kind="ExternalInput")
    wq_d = nc.dram_tensor("wq8", [128, CCH, C], BF16, kind="ExternalInput")
    wk_d = nc.dram_tensor("wk8", [128, CCH, C], F8, kind="ExternalInput")
    wv_d = nc.dram_tensor("wv8", [128, CCH, C], F8, kind="ExternalInput")
    wo_d = nc.dram_tensor("wo8", [128, CCH, C], F32, kind="ExternalInput")
    bq_d = nc.dram_tensor("bq", [128, CCH], F32, kind="ExternalInput")
    bk_d = nc.dram_tensor("bk", [128, CCH], F32, kind="ExternalInput")
    bv_d = nc.dram_tensor("bv", [C], F32, kind="ExternalInput")
    bo_d = nc.dram_tensor("bo", [C], F32, kind="ExternalInput")
    out_d = nc.dram_tensor("out", [NQC * 4, 128, C], F32, kind="ExternalOutput")

    with tile.TileContext(nc) as tc:
        with tc.tile_pool(name="cst", bufs=1) as cst, \
             tc.tile_pool(name="per", bufs=1) as per, \
             tc.tile_pool(name="sb", bufs=1) as sb, \
             tc.tile_pool(name="ps", bufs=1, space="PSUM") as ps:

            # ---- constants / weights (batched DMAs, needed-first order) ----
            ones2 = cst.tile([128, 2, 128], F8)
            nc.gpsimd.memset(ones2[:], 1.0)

            x2t = per.tile([128, CCH, N2], F8, name="x2t", tag="x2t")
            nc.sync.dma_start(out=x2t[:, :, 0:1024], in_=x2t_d[:, :, 0:1024])
            wk8 = cst.tile([128, CCH, C], F8, name="wk8", tag="wk8")
            nc.sync.dma_start(out=wk8[:], in_=wk_d[:])
            wv8 = cst.tile([128, CCH, C], F8, name="wv8", tag="wv8")
            nc.sync.dma_start(out=wv8[:], in_=wv_d[:])
            bk_b = cst.tile([128, CCH], F32, name="bkb", tag="bkb")
            nc.sync.dma_start(out=bk_b[:], in_=bk_d[:])
            bk_t = [bk_b[:, d:d + 1] for d in range(CCH)]
            bv_bc = cst.tile([128, C], F32)
            nc.sync.dma_start(out=bv_bc[:], in_=bv_d[:].unsqueeze(0).broadcast_to([128, C]))
            for blk in range(1, 4):
                nc.sync.dma_start(out=x2t[:, :, blk * 1024:(blk + 1) * 1024],
                                  in_=x2t_d[:, :, blk * 1024:(blk + 1) * 1024])

            wq_b = cst.tile([128, CCH, C], BF16, name="wqb", tag="wqb")
            nc.sync.dma_start(out=wq_b[:], in_=wq_d[:])
            x1t = cst.tile([128, CCH, QROWS], BF16, name="x1tb", tag="x1tb")
            nc.sync.dma_start(out=x1t[:], in_=x1t_d[:])
            bq_b = cst.tile([128, CCH], F32, name="bqb", tag="bqb")
            nc.sync.dma_start(out=bq_b[:], in_=bq_d[:])
            bq_t = [bq_b[:, d:d + 1] for d in range(CCH)]

            wo_stage = sb.tile([128, CCH, C], F32, name="wos", tag="wos", bufs=1)
            nc.sync.dma_start(out=wo_stage[:], in_=wo_d[:])
            wo_t = [cst.tile([128, C], F32R, name=f"wo{d}", tag=f"wo{d}")
                    for d in range(CCH)]
            bo_bc = cst.tile([128, C], F32)
            nc.sync.dma_start(out=bo_bc[:], in_=bo_d[:].unsqueeze(0).broadcast_to([128, C]))

            # ---- persistent attention operands ----
            kt8 = per.tile([128, CCH, N2], F8, name="kt8", tag="kt8")
            v8 = [per.tile([128, 2, C], F8, name=f"v8_{j}", tag=f"v8_{j}")
                  for j in range(KT // 2)]

            # ---- per-chunk state (double buffered across chunks) ----
            def chunk_tiles():
                return {
                    "qtf": [sb.tile([128, QC], F32, name=f"qtf{d}", tag=f"qtf{d}",
                                    bufs=2) for d in range(CCH)],
                    "qt8": sb.tile([128, CCH, QC], F8, name="qt8", tag="qt8", bufs=2),
                    "pt8": [sb.tile([128, 2, QC], F8, name=f"pt{j}", tag=f"pt{j}",
                                    bufs=2) for j in range(KT // 2)],
                    "recip": sb.tile([128, QC], F32, name="recip", tag="recip", bufs=2),
                    "u": [sb.tile([128, QC], F32R, name=f"u{d}", tag=f"u{d}", bufs=2)
                          for d in range(CCH)],
                }

            st = [None] * NQC

            def emit_q(i):
                st[i] = chunk_tiles()
                q0 = i * QC
                for d in range(CCH):
                    qp = ps.tile([128, QC], F32, name="qp", tag="pB", bufs=4)
                    for cc in range(CCH):
                        nc.tensor.matmul(qp[:],
                                         lhsT=wq_b[:, cc, d * 128:(d + 1) * 128],
                                         rhs=x1t[:, cc, q0:q0 + QC],
                                         start=(cc == 0), stop=(cc == CCH - 1))
                    nc.vector.tensor_add(out=st[i]["qtf"][d][:], in0=qp[:],
                                         in1=bq_t[d][:].broadcast_to([128, QC]))
                    nc.scalar.activation(st[i]["qt8"][:, d, :], qp[:], AF.Identity,
                                         bias=bq_t[d][:])

            def emit_x2_group(kc0):
                # K^T d-chunks: casts split DVE (d0,d1) / ACT (d2,d3)
                for d in range(CCH):
                    pp = ps.tile([128, 512], F32, name="kps", tag="pB", bufs=4)
                    for j2 in range(2):
                        nc.tensor.matmul(
                            pp[:],
                            lhsT=wk8[:, 2 * j2:2 * j2 + 2, d * 128:(d + 1) * 128],
                            rhs=x2t[:, 2 * j2:2 * j2 + 2, kc0 * 512:(kc0 + 1) * 512],
                            start=(j2 == 0), stop=(j2 == 1), perf_mode=DR)
                    if d < 2:
                        nc.vector.tensor_add(
                            out=kt8[:, d, kc0 * 512:(kc0 + 1) * 512], in0=pp[:],
                            in1=bk_t[d][:].broadcast_to([128, 512]))
                    else:
                        nc.scalar.activation(kt8[:, d, kc0 * 512:(kc0 + 1) * 512],
                                             pp[:], AF.Identity, bias=bk_t[d][:])
                for kb in range(4):
                    kt = kc0 * 4 + kb
                    pp = ps.tile([128, C], F32, name="vps", tag="pB", bufs=4)
                    for j2 in range(2):
                        nc.tensor.matmul(
                            pp[:],
                            lhsT=x2t[:, 2 * j2:2 * j2 + 2, kt * 128:(kt + 1) * 128],
                            rhs=wv8[:, 2 * j2:2 * j2 + 2, :],
                            start=(j2 == 0), stop=(j2 == 1), perf_mode=DR)
                    nc.vector.tensor_add(out=v8[kt // 2][:, kt % 2, :],
                                         in0=pp[:], in1=bv_bc[:])

            def emit_s_kt(i, kt):
                sp = ps.tile([128, QC], F32, name="sps", tag="pA", bufs=3)
                for j2 in range(2):
                    nc.tensor.matmul(
                        sp[:],
                        lhsT=kt8[:, 2 * j2:2 * j2 + 2, kt * 128:(kt + 1) * 128],
                        rhs=st[i]["qt8"][:, 2 * j2:2 * j2 + 2, :],
                        start=(j2 == 0), stop=(j2 == 1), perf_mode=DR)
                nc.scalar.activation(st[i]["pt8"][kt // 2][:, kt % 2, :], sp[:],
                                     AF.Exp, scale=float(SCALE))

            def emit_r_j(i, j, rs):
                nc.tensor.matmul(rs[:], lhsT=ones2[:], rhs=st[i]["pt8"][j][:],
                                 start=(j == 0), stop=(j == KT // 2 - 1),
                                 perf_mode=DR)

            def emit_pv_j(i, j, ap):
                for d in range(CCH):
                    nc.tensor.matmul(ap[d][:],
                                     lhsT=v8[j][:, :, d * 128:(d + 1) * 128],
                                     rhs=st[i]["pt8"][j][:],
                                     start=(j == 0), stop=(j == KT // 2 - 1),
                                     perf_mode=DR)

            def emit_recip(i, rs):
                nc.vector.reciprocal_approx_fast(out=st[i]["recip"][:], in_=rs[:])

            def emit_u(i, ap):
                for d in range(CCH):
                    at = sb.tile([128, QC], F32, name="at", tag="at", bufs=2)
                    nc.vector.tensor_mul(out=at[:], in0=ap[d][:],
                                         in1=st[i]["recip"][:])
                    nc.vector.tensor_add(out=st[i]["u"][d][:], in0=at[:],
                                         in1=st[i]["qtf"][d][:])

            def emit_o(i):
                ot = sb.tile([128, 4, C], F32, name="ot", tag="ot", bufs=2)
                for rb in range(QC // 128):
                    op = ps.tile([128, C], F32, name="ops", tag="pB", bufs=4)
                    for d in range(CCH):
                        nc.tensor.matmul(
                            op[:],
                            lhsT=st[i]["u"][d][:, rb * 128:(rb + 1) * 128],
                            rhs=wo_t[d][:],
                            start=(d == 0), stop=(d == CCH - 1))
                    nc.vector.tensor_add(out=ot[:, rb, :], in0=op[:], in1=bo_bc[:])
                nc.sync.dma_start(
                    out=out_d[i * 4:(i + 1) * 4, :, :].transpose([1, 0, 2]),
                    in_=ot[:])

            # ---- schedule ----
            # X2 phase with chunk-0 S matmuls interleaved once their
            # K-blocks are ready; Q0 after the first two X2 groups so the
            # x1/wq DMAs hide behind X2 compute
            emit_x2_group(0)
            emit_x2_group(1)
            emit_q(0)
            s0_next = 0
            for kc0 in range(2, N2 // 512):
                emit_x2_group(kc0)
                target = min(3 * (kc0 - 1), 4 * kc0 + 4)
                while s0_next < target:
                    emit_s_kt(0, s0_next)
                    s0_next += 1
            while s0_next < KT:
                emit_s_kt(0, s0_next)
                s0_next += 1
            for d in range(CCH):
                nc.scalar.copy(wo_t[d][:], wo_stage[:, d, :])
            emit_q(1)

            # steady state: chunk i's S interleaved with chunk i-1's
            # rowsum/PV groups (whose inputs are long since ready)
            for i in range(1, NQC):
                rs = ps.tile([128, QC], F32, name="rs", tag="pR", bufs=1)
                ap = [ps.tile([128, QC], F32, name="aps", tag="pB", bufs=4)
                      for _ in range(CCH)]
                for j in range(KT // 2):
                    emit_s_kt(i, 2 * j)
                    emit_s_kt(i, 2 * j + 1)
                    emit_r_j(i - 1, j, rs)
                    emit_pv_j(i - 1, j, ap)
                emit_recip(i - 1, rs)
                emit_u(i - 1, ap)
                if i + 1 < NQC:
                    emit_q(i + 1)
                emit_o(i - 1)

            # last chunk: batched rowsum (exp done by now), PV per-d with
            # immediate normalize+residual so O starts early
            i = NQC - 1
            rs = ps.tile([128, QC], F32, name="rs", tag="pR", bufs=1)
            for j in range(KT // 2):
                emit_r_j(i, j, rs)
            emit_recip(i, rs)
            for d in range(CCH):
                apd = ps.tile([128, QC], F32, name="aps", tag="pB", bufs=4)
                for j in range(KT // 2):
                    nc.tensor.matmul(apd[:],
                                     lhsT=v8[j][:, :, d * 128:(d + 1) * 128],
                                     rhs=st[i]["pt8"][j][:],
                                     start=(j == 0), stop=(j == KT // 2 - 1),
                                     perf_mode=DR)
                at = sb.tile([128, QC], F32, name="at", tag="at", bufs=2)
                nc.vector.tensor_mul(out=at[:], in0=apd[:], in1=st[i]["recip"][:])
                nc.vector.tensor_add(out=st[i]["u"][d][:], in0=at[:],
                                     in1=st[i]["qtf"][d][:])
            emit_o(i)

    nc.compile()
    return nc


def get_built():
    global _BUILT
    if _BUILT is None:
        _BUILT = build()
    return _BUILT


def _pair_layout(a, dt):
    # [512 (contract), X] -> [128, 4, X]: [p, j, x] = a[j*128+p, x]
    return np.ascontiguousarray(
        a.reshape(CCH, 128, -1).transpose(1, 0, 2)).astype(dt)


def make_in_maps(x1, x2, Wq, bq, Wk, bk, Wv, bv, Wo, bo):
    bf = ml_dtypes.bfloat16
    f8 = ml_dtypes.float8_e4m3
    wq8 = _pair_layout(np.ascontiguousarray(Wq.T), bf)
    wk8 = _pair_layout(np.ascontiguousarray(Wk.T), f8)
    wv8 = _pair_layout(np.ascontiguousarray(Wv.T), f8)
    wo8 = _pair_layout(np.ascontiguousarray(Wo.T), np.float32)
    bq32 = np.ascontiguousarray(bq.reshape(CCH, 128).T).astype(np.float32)
    bk32 = np.ascontiguousarray(bk.reshape(CCH, 128).T).astype(np.float32)
    bv32 = bv.astype(np.float32)
    bo32 = bo.astype(np.float32)
    x2t8 = [_pair_layout(np.ascontiguousarray(x2[b].T), f8) for b in range(B)]
    in_maps = []
    for cid in range(NCORES):
        b, h = cid // 2, cid % 2
        x1s = x1[b, h * QROWS:(h + 1) * QROWS, :]
        in_maps.append({
            "x1t": _pair_layout(np.ascontiguousarray(x1s.T), bf),
            "x2t": x2t8[b],
            "wq8": wq8, "wk8": wk8, "wv8": wv8, "wo8": wo8,
            "bq": bq32, "bk": bk32, "bv": bv32, "bo": bo32,
        })
    return in_maps


LAST_RESULT = None


def kernel(x1, x2, Wq, bq, Wk, bk, Wv, bv, Wo, bo):
    global LAST_RESULT
    nc = get_built()
    in_maps = make_in_maps(x1, x2, Wq, bq, Wk, bk, Wv, bv, Wo, bo)
    trace = bool(os.environ.get("KERNEL_TRACE"))
    res = run_bass_kernel_spmd(nc, in_maps, core_ids=list(range(NCORES)), trace=trace)
    LAST_RESULT = res
    out = np.empty((B, N1, C), dtype=np.float32)
    for cid in range(NCORES):
        b, h = cid // 2, cid % 2
        out[b, h * QROWS:(h + 1) * QROWS, :] = \
            res.results[cid]["out"].reshape(QROWS, C)
    return out


# revision 30
# speedup vs baseline: 2.3898x; 1.0132x over previous
"""CrossFeatureAttention TRN2 kernel (fp8 DoubleRow attention).

Full inputs -> full output. Sharding: data-parallel over (batch b, half of N1)
across 8 cores; each core computes out[b, h*2048:(h+1)*2048, :].

Math (per core, x1 slice q=2048 rows, x2[b] k=4096 rows, C=512):
    Q  = x1 @ Wq^T + bq              (bf16 matmul, fp32 kept for the residual)
    K^T= Wk x2^T + bk                (fp8 DoubleRow)
    V  = x2 @ Wv^T + bv              (fp8 DoubleRow)
    P  = exp(Q K^T / sqrt(C))        (fp8 DoubleRow scores, ACT exp -> fp8)
    rs = colsum(P^T)                 (ones-matmul, fp8 DoubleRow)
    U^T= Q^T + (V^T P^T) * recip(rs) (PV fp8 DoubleRow, fp32 combine on DVE)
    out= U @ Wo^T + bo               (f32r matmul at full rate)

All fp8 matmuls use MatmulPerfMode.DoubleRow: operands hold contraction
k-tile PAIRS in a [128, 2, F] layout (dim1 = k-tile index), halving PE
cycles vs bf16.  The residual path stays fp32: Q^T is kept from PSUM and
added to the normalized attention output on DVE, and the output projection
runs with float32r operands (1 cycle/row at free-dim 512).

Chunks of 512 q-rows are software-pipelined: chunk i+1's Q-projection and
S-matmuls are issued before chunk i's rowsum/PV/out so the ACT exp tail of
chunk i hides behind PE work.
"""

import os
import sys

import numpy as np

for _p in ("/root/.axon_site", "/root/.axon_site/_ro/trn_rl_repo",
           "/root/.axon_site/_ro/pypackages"):
    if _p not in sys.path and os.path.isdir(_p):
        sys.path.append(_p)

import ml_dtypes

import concourse.bacc as bacc
import concourse.mybir as mybir
import concourse.tile as tile
from concourse.bass_utils import run_bass_kernel_spmd

F32 = mybir.dt.float32
F32R = mybir.dt.float32r
BF16 = mybir.dt.bfloat16
F8 = mybir.dt.float8e4
AF = mybir.ActivationFunctionType
DR = mybir.MatmulPerfMode.DoubleRow

B, N1, N2, C = 4, 4096, 4096, 512
NCORES = 8
QROWS = N1 * B // NCORES          # 2048 q rows per core
QC = 512                          # q-chunk
NQC = QROWS // QC                 # 4 chunks
KT = N2 // 128                    # 32 k-tiles
CCH = C // 128                    # 4 contraction chunks
SCALE = 1.0 / float(np.sqrt(C))

_BUILT = None


def build():
    nc = bacc.Bacc(None, target_bir_lowering=False, debug=False)

    x1t_d = nc.dram_tensor("x1t", [128, CCH, QROWS], BF16, kind="ExternalInput")
    x2t_d = nc.dram_tensor("x2t", [128, CCH, N2], F8, kind="ExternalInput")
    wq_d = nc.dram_tensor("wq8", [128, CCH, C], BF16, kind="ExternalInput")
    wk_d = nc.dram_tensor("wk8", [128, CCH, C], F8, kind="ExternalInput")
    wv_d = nc.dram_tensor("wv8", [128, CCH, C], F8, kind="ExternalInput")
    wo_d = nc.dram_tensor("wo8", [128, CCH, C], F32, kind="ExternalInput")
    bq_d = nc.dram_tensor("bq", [128, CCH], F32, kind="ExternalInput")
    bk_d = nc.dram_tensor("bk", [128, CCH], F32, kind="ExternalInput")
    bv_d = nc.dram_tensor("bv", [C], F32, kind="ExternalInput")
    bo_d = nc.dram_tensor("bo", [C], F32, kind="ExternalInput")
    out_d = nc.dram_tensor("out", [NQC * 4, 128, C], F32, kind="ExternalOutput")

    with tile.TileContext(nc) as tc:
        with tc.tile_pool(name="cst", bufs=1) as cst, \
             tc.tile_pool(name="per", bufs=1) as per, \
             tc.tile_pool(name="sb", bufs=1) as sb, \
             tc.tile_pool(name="ps", bufs=1, space="PSUM") as ps:

            # ---- constants / weights (batched DMAs, needed-first order) ----
            ones2 = cst.tile([128, 2, 128], F8)
            nc.gpsimd.memset(ones2[:], 1.0)

            wk8 = cst.tile([128, CCH, C], F8, name="wk8", tag="wk8")
            nc.sync.dma_start(out=wk8[:], in_=wk_d[:])
            wv8 = cst.tile([128, CCH, C], F8, name="wv8", tag="wv8")
            nc.sync.dma_start(out=wv8[:], in_=wv_d[:])
            x2t = per.tile([128, CCH, N2], F8, name="x2t", tag="x2t")
            nc.sync.dma_start(out=x2t[:, :, 0:512], in_=x2t_d[:, :, 0:512])
            bk_b = cst.tile([128, CCH], F32, name="bkb", tag="bkb")
            nc.sync.dma_start(out=bk_b[:], in_=bk_d[:])
            bk_t = [bk_b[:, d:d + 1] for d in range(CCH)]
            bv_bc = cst.tile([128, C], F32)
            nc.sync.dma_start(out=bv_bc[:], in_=bv_d[:].unsqueeze(0).broadcast_to([128, C]))
            for lo, hi in ((512, 1536), (1536, 2560), (2560, 4096)):
                nc.sync.dma_start(out=x2t[:, :, lo:hi], in_=x2t_d[:, :, lo:hi])

            wq_b = cst.tile([128, CCH, C], BF16, name="wqb", tag="wqb")
            nc.sync.dma_start(out=wq_b[:], in_=wq_d[:])
            x1t = cst.tile([128, CCH, QROWS], BF16, name="x1tb", tag="x1tb")
            nc.sync.dma_start(out=x1t[:], in_=x1t_d[:])
            bq_b = cst.tile([128, CCH], F32, name="bqb", tag="bqb")
            nc.sync.dma_start(out=bq_b[:], in_=bq_d[:])
            bq_t = [bq_b[:, d:d + 1] for d in range(CCH)]

            wo_stage = sb.tile([128, CCH, C], F32, name="wos", tag="wos", bufs=1)
            nc.sync.dma_start(out=wo_stage[:], in_=wo_d[:])
            wo_t = [cst.tile([128, C], F32R, name=f"wo{d}", tag=f"wo{d}")
                    for d in range(CCH)]
            bo_bc = cst.tile([128, C], F32)
            nc.sync.dma_start(out=bo_bc[:], in_=bo_d[:].unsqueeze(0).broadcast_to([128, C]))

            # ---- persistent attention operands ----
            kt8 = per.tile([128, CCH, N2], F8, name="kt8", tag="kt8")
            v8 = [per.tile([128, 2, C], F8, name=f"v8_{j}", tag=f"v8_{j}")
                  for j in range(KT // 2)]

            # ---- per-chunk state (double buffered across chunks) ----
            def chunk_tiles():
                return {
                    "qtf": [sb.tile([128, QC], F32, name=f"qtf{d}", tag=f"qtf{d}",
                                    bufs=2) for d in range(CCH)],
                    "qt8": sb.tile([128, CCH, QC], F8, name="qt8", tag="qt8", bufs=2),
                    "pt8": [sb.tile([128, 2, QC], F8, name=f"pt{j}", tag=f"pt{j}",
                                    bufs=2) for j in range(KT // 2)],
                    "recip": sb.tile([128, QC], F32, name="recip", tag="recip", bufs=2),
                    "u": [sb.tile([128, QC], F32R, name=f"u{d}", tag=f"u{d}", bufs=2)
                          for d in range(CCH)],
                }

            st = [None] * NQC

            def emit_q(i):
                st[i] = chunk_tiles()
                q0 = i * QC
                for d in range(CCH):
                    qp = ps.tile([128, QC], F32, name="qp", tag="pR", bufs=2)
                    for cc in range(CCH):
                        nc.tensor.matmul(qp[:],
                                         lhsT=wq_b[:, cc, d * 128:(d + 1) * 128],
                                         rhs=x1t[:, cc, q0:q0 + QC],
                                         start=(cc == 0), stop=(cc == CCH - 1))
                    nc.vector.tensor_add(out=st[i]["qtf"][d][:], in0=qp[:],
                                         in1=bq_t[d][:].broadcast_to([128, QC]))
                    nc.scalar.activation(st[i]["qt8"][:, d, :], qp[:], AF.Identity,
                                         bias=bq_t[d][:])

            def emit_x2_group(kc0):
                # K^T d-chunks: casts split DVE (d0,d1) / ACT (d2,d3)
                for d in range(CCH):
                    pp = ps.tile([128, 512], F32, name="kps", tag="pB", bufs=4)
                    for j2 in range(2):
                        nc.tensor.matmul(
                            pp[:],
                            lhsT=wk8[:, 2 * j2:2 * j2 + 2, d * 128:(d + 1) * 128],
                            rhs=x2t[:, 2 * j2:2 * j2 + 2, kc0 * 512:(kc0 + 1) * 512],
                            start=(j2 == 0), stop=(j2 == 1), perf_mode=DR)
                    if d < 2:
                        nc.vector.tensor_add(
                            out=kt8[:, d, kc0 * 512:(kc0 + 1) * 512], in0=pp[:],
                            in1=bk_t[d][:].broadcast_to([128, 512]))
                    else:
                        nc.scalar.activation(kt8[:, d, kc0 * 512:(kc0 + 1) * 512],
                                             pp[:], AF.Identity, bias=bk_t[d][:])
                for kb in range(4):
                    kt = kc0 * 4 + kb
                    pp = ps.tile([128, C], F32, name="vps", tag="pB", bufs=4)
                    for j2 in range(2):
                        nc.tensor.matmul(
                            pp[:],
                            lhsT=x2t[:, 2 * j2:2 * j2 + 2, kt * 128:(kt + 1) * 128],
                            rhs=wv8[:, 2 * j2:2 * j2 + 2, :],
                            start=(j2 == 0), stop=(j2 == 1), perf_mode=DR)
                    nc.vector.tensor_add(out=v8[kt // 2][:, kt % 2, :],
                                         in0=pp[:], in1=bv_bc[:])

            def emit_s_kt(i, kt):
                sp = ps.tile([128, QC], F32, name="sps", tag="pA", bufs=2)
                for j2 in range(2):
                    nc.tensor.matmul(
                        sp[:],
                        lhsT=kt8[:, 2 * j2:2 * j2 + 2, kt * 128:(kt + 1) * 128],
                        rhs=st[i]["qt8"][:, 2 * j2:2 * j2 + 2, :],
                        start=(j2 == 0), stop=(j2 == 1), perf_mode=DR)
                nc.scalar.activation(st[i]["pt8"][kt // 2][:, kt % 2, :], sp[:],
                                     AF.Exp, scale=float(SCALE))

            def emit_r_j(i, j, rs):
                nc.tensor.matmul(rs[:], lhsT=ones2[:], rhs=st[i]["pt8"][j][:],
                                 start=(j == 0), stop=(j == KT // 2 - 1),
                                 perf_mode=DR)

            def emit_pv_j(i, j, ap):
                for d in range(CCH):
                    nc.tensor.matmul(ap[d][:],
                                     lhsT=v8[j][:, :, d * 128:(d + 1) * 128],
                                     rhs=st[i]["pt8"][j][:],
                                     start=(j == 0), stop=(j == KT // 2 - 1),
                                     perf_mode=DR)

            def emit_recip(i, rs):
                nc.vector.reciprocal_approx_fast(out=st[i]["recip"][:], in_=rs[:])

            def emit_u(i, ap):
                for d in range(CCH):
                    at = sb.tile([128, QC], F32, name="at", tag="at", bufs=2)
                    nc.vector.tensor_mul(out=at[:], in0=ap[d][:],
                                         in1=st[i]["recip"][:])
                    nc.vector.tensor_add(out=st[i]["u"][d][:], in0=at[:],
                                         in1=st[i]["qtf"][d][:])

            def emit_o(i):
                for rb in range(QC // 128):
                    op = ps.tile([128, C], F32, name="ops", tag="pB", bufs=4)
                    for d in range(CCH):
                        nc.tensor.matmul(
                            op[:],
                            lhsT=st[i]["u"][d][:, rb * 128:(rb + 1) * 128],
                            rhs=wo_t[d][:],
                            start=(d == 0), stop=(d == CCH - 1))
                    ot = sb.tile([128, C], F32, name="ot", tag="ot", bufs=3)
                    nc.vector.tensor_add(out=ot[:], in0=op[:], in1=bo_bc[:])
                    nc.sync.dma_start(out=out_d[i * 4 + rb, :, :], in_=ot[:])

            # ---- schedule ----
            # X2 phase with chunk-0 S matmuls interleaved once their
            # K-blocks are ready; Q0 after the first two X2 groups so the
            # x1/wq DMAs hide behind X2 compute
            emit_x2_group(0)
            emit_x2_group(1)
            emit_q(0)
            s0_next = 0
            for kc0 in range(2, N2 // 512):
                emit_x2_group(kc0)
                target = min(3 * (kc0 - 1), 4 * kc0 + 4)
                while s0_next < target:
                    emit_s_kt(0, s0_next)
                    s0_next += 1
            while s0_next < KT:
                emit_s_kt(0, s0_next)
                s0_next += 1
            for d in range(CCH):
                nc.scalar.copy(wo_t[d][:], wo_stage[:, d, :])
            emit_q(1)

            # steady state: chunk i's S interleaved with chunk i-1's
            # rowsum/PV groups (whose inputs are long since ready)
            for i in range(1, NQC):
                rs = ps.tile([128, QC], F32, name="rs", tag="pR", bufs=2)
                ap = [ps.tile([128, QC], F32, name="aps", tag="pB", bufs=4)
                      for _ in range(CCH)]
                for j in range(KT // 2):
                    emit_s_kt(i, 2 * j)
                    emit_s_kt(i, 2 * j + 1)
                    emit_r_j(i - 1, j, rs)
                    emit_pv_j(i - 1, j, ap)
                emit_recip(i - 1, rs)
                emit_u(i - 1, ap)
                if i + 1 < NQC:
                    emit_q(i + 1)
                emit_o(i - 1)

            # last chunk: batched rowsum (exp done by now), PV per-d with
            # immediate normalize+residual so O starts early
            i = NQC - 1
            rs = ps.tile([128, QC], F32, name="rs", tag="pR", bufs=2)
            for j in range(KT // 2):
                emit_r_j(i, j, rs)
            emit_recip(i, rs)
            for d in range(CCH):
                apd = ps.tile([128, QC], F32, name="aps", tag="pB", bufs=4)
                for j in range(KT // 2):
                    nc.tensor.matmul(apd[:],
                                     lhsT=v8[j][:, :, d * 128:(d + 1) * 128],
                                     rhs=st[i]["pt8"][j][:],
                                     start=(j == 0), stop=(j == KT // 2 - 1),
                                     perf_mode=DR)
                at = sb.tile([128, QC], F32, name="at", tag="at", bufs=2)
                nc.vector.tensor_mul(out=at[:], in0=apd[:], in1=st[i]["recip"][:])
                nc.vector.tensor_add(out=st[i]["u"][d][:], in0=at[:],
                                     in1=st[i]["qtf"][d][:])
            emit_o(i)

    nc.compile()
    return nc


def get_built():
    global _BUILT
    if _BUILT is None:
        _BUILT = build()
    return _BUILT


def _pair_layout(a, dt):
    # [512 (contract), X] -> [128, 4, X]: [p, j, x] = a[j*128+p, x]
    return np.ascontiguousarray(
        a.reshape(CCH, 128, -1).transpose(1, 0, 2)).astype(dt)


def make_in_maps(x1, x2, Wq, bq, Wk, bk, Wv, bv, Wo, bo):
    bf = ml_dtypes.bfloat16
    f8 = ml_dtypes.float8_e4m3
    wq8 = _pair_layout(np.ascontiguousarray(Wq.T), bf)
    wk8 = _pair_layout(np.ascontiguousarray(Wk.T), f8)
    wv8 = _pair_layout(np.ascontiguousarray(Wv.T), f8)
    wo8 = _pair_layout(np.ascontiguousarray(Wo.T), np.float32)
    bq32 = np.ascontiguousarray(bq.reshape(CCH, 128).T).astype(np.float32)
    bk32 = np.ascontiguousarray(bk.reshape(CCH, 128).T).astype(np.float32)
    bv32 = bv.astype(np.float32)
    bo32 = bo.astype(np.float32)
    x2t8 = [_pair_layout(np.ascontiguousarray(x2[b].T), f8) for b in range(B)]
    in_maps = []
    for cid in range(NCORES):
        b, h = cid // 2, cid % 2
        x1s = x1[b, h * QROWS:(h + 1) * QROWS, :]
        in_maps.append({
            "x1t": _pair_layout(np.ascontiguousarray(x1s.T), bf),
            "x2t": x2t8[b],
            "wq8": wq8, "wk8": wk8, "wv8": wv8, "wo8": wo8,
            "bq": bq32, "bk": bk32, "bv": bv32, "bo": bo32,
        })
    return in_maps


LAST_RESULT = None


def kernel(x1, x2, Wq, bq, Wk, bk, Wv, bv, Wo, bo):
    global LAST_RESULT
    nc = get_built()
    in_maps = make_in_maps(x1, x2, Wq, bq, Wk, bk, Wv, bv, Wo, bo)
    trace = bool(os.environ.get("KERNEL_TRACE"))
    res = run_bass_kernel_spmd(nc, in_maps, core_ids=list(range(NCORES)), trace=trace)
    LAST_RESULT = res
    out = np.empty((B, N1, C), dtype=np.float32)
    for cid in range(NCORES):
        b, h = cid // 2, cid % 2
        out[b, h * QROWS:(h + 1) * QROWS, :] = \
            res.results[cid]["out"].reshape(QROWS, C)
    return out


# revision 32
# speedup vs baseline: 2.3992x; 1.0039x over previous
"""CrossFeatureAttention TRN2 kernel (fp8 DoubleRow attention).

Full inputs -> full output. Sharding: data-parallel over (batch b, half of N1)
across 8 cores; each core computes out[b, h*2048:(h+1)*2048, :].

Math (per core, x1 slice q=2048 rows, x2[b] k=4096 rows, C=512):
    Q  = x1 @ Wq^T + bq              (bf16 matmul, fp32 kept for the residual)
    K^T= Wk x2^T + bk                (fp8 DoubleRow)
    V  = x2 @ Wv^T + bv              (fp8 DoubleRow)
    P  = exp(Q K^T / sqrt(C))        (fp8 DoubleRow scores, ACT exp -> fp8)
    rs = colsum(P^T)                 (ones-matmul, fp8 DoubleRow)
    U^T= Q^T + (V^T P^T) * recip(rs) (PV fp8 DoubleRow, fp32 combine on DVE)
    out= U @ Wo^T + bo               (f32r matmul at full rate)

All fp8 matmuls use MatmulPerfMode.DoubleRow: operands hold contraction
k-tile PAIRS in a [128, 2, F] layout (dim1 = k-tile index), halving PE
cycles vs bf16.  The residual path stays fp32: Q^T is kept from PSUM and
added to the normalized attention output on DVE, and the output projection
runs with float32r operands (1 cycle/row at free-dim 512).

Chunks of 512 q-rows are software-pipelined: chunk i+1's Q-projection and
S-matmuls are issued before chunk i's rowsum/PV/out so the ACT exp tail of
chunk i hides behind PE work.
"""

import os
import sys

import numpy as np

for _p in ("/root/.axon_site", "/root/.axon_site/_ro/trn_rl_repo",
           "/root/.axon_site/_ro/pypackages"):
    if _p not in sys.path and os.path.isdir(_p):
        sys.path.append(_p)

import ml_dtypes

import concourse.bacc as bacc
import concourse.mybir as mybir
import concourse.tile as tile
from concourse.bass_utils import run_bass_kernel_spmd

F32 = mybir.dt.float32
F32R = mybir.dt.float32r
BF16 = mybir.dt.bfloat16
F8 = mybir.dt.float8e4
AF = mybir.ActivationFunctionType
DR = mybir.MatmulPerfMode.DoubleRow

B, N1, N2, C = 4, 4096, 4096, 512
NCORES = 8
QROWS = N1 * B // NCORES          # 2048 q rows per core
QC = 512                          # q-chunk
NQC = QROWS // QC                 # 4 chunks
KT = N2 // 128                    # 32 k-tiles
CCH = C // 128                    # 4 contraction chunks
SCALE = 1.0 / float(np.sqrt(C))

_BUILT = None


def build():
    nc = bacc.Bacc(None, target_bir_lowering=False, debug=False)

    x1t_d = nc.dram_tensor("x1t", [128, CCH, QROWS], BF16, kind="ExternalInput")
    x2t_d = nc.dram_tensor("x2t", [128, CCH, N2], F8, kind="ExternalInput")
    wq_d = nc.dram_tensor("wq8", [128, CCH, C], BF16, kind="ExternalInput")
    wk_d = nc.dram_tensor("wk8", [128, CCH, C], F8, kind="ExternalInput")
    wv_d = nc.dram_tensor("wv8", [128, CCH, C], F8, kind="ExternalInput")
    wo_d = nc.dram_tensor("wo8", [128, CCH, C], F32, kind="ExternalInput")
    bq_d = nc.dram_tensor("bq", [128, CCH], F32, kind="ExternalInput")
    bk_d = nc.dram_tensor("bk", [128, CCH], F32, kind="ExternalInput")
    bv_d = nc.dram_tensor("bv", [C], F32, kind="ExternalInput")
    bo_d = nc.dram_tensor("bo", [C], F32, kind="ExternalInput")
    out_d = nc.dram_tensor("out", [NQC * 4, 128, C], F32, kind="ExternalOutput")

    with tile.TileContext(nc) as tc:
        with tc.tile_pool(name="cst", bufs=1) as cst, \
             tc.tile_pool(name="per", bufs=1) as per, \
             tc.tile_pool(name="sb", bufs=1) as sb, \
             tc.tile_pool(name="ps", bufs=1, space="PSUM") as ps:

            # ---- constants / weights (batched DMAs, needed-first order) ----
            ones2 = cst.tile([128, 2, 128], F8)
            nc.gpsimd.memset(ones2[:], 1.0)
            warm = cst.tile([128, 2, 512], F8, name="warm", tag="warm")
            nc.gpsimd.memset(warm[:], 0.0)

            wk8 = cst.tile([128, CCH, C], F8, name="wk8", tag="wk8")
            nc.sync.dma_start(out=wk8[:], in_=wk_d[:])
            wv8 = cst.tile([128, CCH, C], F8, name="wv8", tag="wv8")
            nc.sync.dma_start(out=wv8[:], in_=wv_d[:])
            x2t = per.tile([128, CCH, N2], F8, name="x2t", tag="x2t")
            nc.sync.dma_start(out=x2t[:, :, 0:512], in_=x2t_d[:, :, 0:512])
            bk_b = cst.tile([128, CCH], F32, name="bkb", tag="bkb")
            nc.sync.dma_start(out=bk_b[:], in_=bk_d[:])
            bk_t = [bk_b[:, d:d + 1] for d in range(CCH)]
            bv_bc = cst.tile([128, C], F32)
            nc.sync.dma_start(out=bv_bc[:], in_=bv_d[:].unsqueeze(0).broadcast_to([128, C]))
            for lo, hi in ((512, 1536), (1536, 2560), (2560, 4096)):
                nc.sync.dma_start(out=x2t[:, :, lo:hi], in_=x2t_d[:, :, lo:hi])

            wq_b = cst.tile([128, CCH, C], BF16, name="wqb", tag="wqb")
            nc.sync.dma_start(out=wq_b[:], in_=wq_d[:])
            x1t = cst.tile([128, CCH, QROWS], BF16, name="x1tb", tag="x1tb")
            nc.sync.dma_start(out=x1t[:], in_=x1t_d[:])
            bq_b = cst.tile([128, CCH], F32, name="bqb", tag="bqb")
            nc.sync.dma_start(out=bq_b[:], in_=bq_d[:])
            bq_t = [bq_b[:, d:d + 1] for d in range(CCH)]

            wo_stage = sb.tile([128, CCH, C], F32, name="wos", tag="wos", bufs=1)
            nc.sync.dma_start(out=wo_stage[:], in_=wo_d[:])
            wo_t = [cst.tile([128, C], F32R, name=f"wo{d}", tag=f"wo{d}")
                    for d in range(CCH)]
            bo_bc = cst.tile([128, C], F32)
            nc.sync.dma_start(out=bo_bc[:], in_=bo_d[:].unsqueeze(0).broadcast_to([128, C]))

            # ---- persistent attention operands ----
            kt8 = per.tile([128, CCH, N2], F8, name="kt8", tag="kt8")
            v8 = [per.tile([128, 2, C], F8, name=f"v8_{j}", tag=f"v8_{j}")
                  for j in range(KT // 2)]

            # ---- per-chunk state (double buffered across chunks) ----
            def chunk_tiles():
                return {
                    "qtf": [sb.tile([128, QC], F32, name=f"qtf{d}", tag=f"qtf{d}",
                                    bufs=2) for d in range(CCH)],
                    "qt8": sb.tile([128, CCH, QC], F8, name="qt8", tag="qt8", bufs=2),
                    "pt8": [sb.tile([128, 2, QC], F8, name=f"pt{j}", tag=f"pt{j}",
                                    bufs=2) for j in range(KT // 2)],
                    "recip": sb.tile([128, QC], F32, name="recip", tag="recip", bufs=2),
                    "u": [sb.tile([128, QC], F32R, name=f"u{d}", tag=f"u{d}", bufs=2)
                          for d in range(CCH)],
                }

            st = [None] * NQC

            def emit_q(i):
                st[i] = chunk_tiles()
                q0 = i * QC
                for d in range(CCH):
                    qp = ps.tile([128, QC], F32, name="qp", tag="pR", bufs=2)
                    for cc in range(CCH):
                        nc.tensor.matmul(qp[:],
                                         lhsT=wq_b[:, cc, d * 128:(d + 1) * 128],
                                         rhs=x1t[:, cc, q0:q0 + QC],
                                         start=(cc == 0), stop=(cc == CCH - 1))
                    nc.vector.tensor_add(out=st[i]["qtf"][d][:], in0=qp[:],
                                         in1=bq_t[d][:].broadcast_to([128, QC]))
                    nc.scalar.activation(st[i]["qt8"][:, d, :], qp[:], AF.Identity,
                                         bias=bq_t[d][:])

            def emit_x2_group(kc0):
                # K^T d-chunks: casts split DVE (d0,d1) / ACT (d2,d3)
                for d in range(CCH):
                    pp = ps.tile([128, 512], F32, name="kps", tag="pB", bufs=4)
                    for j2 in range(2):
                        nc.tensor.matmul(
                            pp[:],
                            lhsT=wk8[:, 2 * j2:2 * j2 + 2, d * 128:(d + 1) * 128],
                            rhs=x2t[:, 2 * j2:2 * j2 + 2, kc0 * 512:(kc0 + 1) * 512],
                            start=(j2 == 0), stop=(j2 == 1), perf_mode=DR)
                    if d < 2:
                        nc.vector.tensor_add(
                            out=kt8[:, d, kc0 * 512:(kc0 + 1) * 512], in0=pp[:],
                            in1=bk_t[d][:].broadcast_to([128, 512]))
                    else:
                        nc.scalar.activation(kt8[:, d, kc0 * 512:(kc0 + 1) * 512],
                                             pp[:], AF.Identity, bias=bk_t[d][:])
                for kb in range(4):
                    kt = kc0 * 4 + kb
                    pp = ps.tile([128, C], F32, name="vps", tag="pB", bufs=4)
                    for j2 in range(2):
                        nc.tensor.matmul(
                            pp[:],
                            lhsT=x2t[:, 2 * j2:2 * j2 + 2, kt * 128:(kt + 1) * 128],
                            rhs=wv8[:, 2 * j2:2 * j2 + 2, :],
                            start=(j2 == 0), stop=(j2 == 1), perf_mode=DR)
                    nc.vector.tensor_add(out=v8[kt // 2][:, kt % 2, :],
                                         in0=pp[:], in1=bv_bc[:])

            def emit_s_kt(i, kt):
                sp = ps.tile([128, QC], F32, name="sps", tag="pA", bufs=2)
                for j2 in range(2):
                    nc.tensor.matmul(
                        sp[:],
                        lhsT=kt8[:, 2 * j2:2 * j2 + 2, kt * 128:(kt + 1) * 128],
                        rhs=st[i]["qt8"][:, 2 * j2:2 * j2 + 2, :],
                        start=(j2 == 0), stop=(j2 == 1), perf_mode=DR)
                nc.scalar.activation(st[i]["pt8"][kt // 2][:, kt % 2, :], sp[:],
                                     AF.Exp, scale=float(SCALE))

            def emit_r_j(i, j, rs):
                nc.tensor.matmul(rs[:], lhsT=ones2[:], rhs=st[i]["pt8"][j][:],
                                 start=(j == 0), stop=(j == KT // 2 - 1),
                                 perf_mode=DR)

            def emit_pv_j(i, j, ap):
                for d in range(CCH):
                    nc.tensor.matmul(ap[d][:],
                                     lhsT=v8[j][:, :, d * 128:(d + 1) * 128],
                                     rhs=st[i]["pt8"][j][:],
                                     start=(j == 0), stop=(j == KT // 2 - 1),
                                     perf_mode=DR)

            def emit_recip(i, rs):
                nc.vector.reciprocal_approx_fast(out=st[i]["recip"][:], in_=rs[:])

            def emit_u(i, ap):
                for d in range(CCH):
                    at = sb.tile([128, QC], F32, name="at", tag="at", bufs=2)
                    nc.vector.tensor_mul(out=at[:], in0=ap[d][:],
                                         in1=st[i]["recip"][:])
                    nc.vector.tensor_add(out=st[i]["u"][d][:], in0=at[:],
                                         in1=st[i]["qtf"][d][:])

            def emit_o(i):
                for rb in range(QC // 128):
                    op = ps.tile([128, C], F32, name="ops", tag="pB", bufs=4)
                    for d in range(CCH):
                        nc.tensor.matmul(
                            op[:],
                            lhsT=st[i]["u"][d][:, rb * 128:(rb + 1) * 128],
                            rhs=wo_t[d][:],
                            start=(d == 0), stop=(d == CCH - 1))
                    ot = sb.tile([128, C], F32, name="ot", tag="ot", bufs=3)
                    nc.vector.tensor_add(out=ot[:], in0=op[:], in1=bo_bc[:])
                    nc.sync.dma_start(out=out_d[i * 4 + rb, :, :], in_=ot[:])

            # ---- schedule ----
            # PE warmup during the DMA lead-in: ramps the clock before real
            # work and keeps HAM from starting the kernel cold
            for w in range(10):
                wp = ps.tile([128, QC], F32, name="warmp", tag="pA", bufs=2)
                nc.tensor.matmul(wp[:], lhsT=ones2[:], rhs=warm[:],
                                 start=True, stop=True, perf_mode=DR)
            # X2 phase with chunk-0 S matmuls interleaved once their
            # K-blocks are ready; Q0 after the first two X2 groups so the
            # x1/wq DMAs hide behind X2 compute
            emit_x2_group(0)
            emit_x2_group(1)
            emit_q(0)
            s0_next = 0
            for kc0 in range(2, N2 // 512):
                emit_x2_group(kc0)
                target = min(3 * (kc0 - 1), 4 * kc0 + 4)
                while s0_next < target:
                    emit_s_kt(0, s0_next)
                    s0_next += 1
            while s0_next < KT:
                emit_s_kt(0, s0_next)
                s0_next += 1
            for d in range(CCH):
                nc.scalar.copy(wo_t[d][:], wo_stage[:, d, :])
            emit_q(1)

            # steady state: chunk i's S interleaved with chunk i-1's
            # rowsum/PV groups (whose inputs are long since ready)
            for i in range(1, NQC):
                rs = ps.tile([128, QC], F32, name="rs", tag="pR", bufs=2)
                ap = [ps.tile([128, QC], F32, name="aps", tag="pB", bufs=4)
                      for _ in range(CCH)]
                for j in range(KT // 2):
                    emit_s_kt(i, 2 * j)
                    emit_s_kt(i, 2 * j + 1)
                    emit_r_j(i - 1, j, rs)
                    emit_pv_j(i - 1, j, ap)
                emit_recip(i - 1, rs)
                emit_u(i - 1, ap)
                if i + 1 < NQC:
                    emit_q(i + 1)
                emit_o(i - 1)

            # last chunk: batched rowsum (exp done by now), PV per-d with
            # immediate normalize+residual so O starts early
            i = NQC - 1
            rs = ps.tile([128, QC], F32, name="rs", tag="pR", bufs=2)
            for j in range(KT // 2):
                emit_r_j(i, j, rs)
            emit_recip(i, rs)
            for d in range(CCH):
                apd = ps.tile([128, QC], F32, name="aps", tag="pB", bufs=4)
                for j in range(KT // 2):
                    nc.tensor.matmul(apd[:],
                                     lhsT=v8[j][:, :, d * 128:(d + 1) * 128],
                                     rhs=st[i]["pt8"][j][:],
                                     start=(j == 0), stop=(j == KT // 2 - 1),
                                     perf_mode=DR)
                at = sb.tile([128, QC], F32, name="at", tag="at", bufs=2)
                nc.vector.tensor_mul(out=at[:], in0=apd[:], in1=st[i]["recip"][:])
                nc.vector.tensor_add(out=st[i]["u"][d][:], in0=at[:],
                                     in1=st[i]["qtf"][d][:])
            emit_o(i)

    nc.compile()
    return nc


def get_built():
    global _BUILT
    if _BUILT is None:
        _BUILT = build()
    return _BUILT


def _pair_layout(a, dt):
    # [512 (contract), X] -> [128, 4, X]: [p, j, x] = a[j*128+p, x]
    return np.ascontiguousarray(
        a.reshape(CCH, 128, -1).transpose(1, 0, 2)).astype(dt)


def make_in_maps(x1, x2, Wq, bq, Wk, bk, Wv, bv, Wo, bo):
    bf = ml_dtypes.bfloat16
    f8 = ml_dtypes.float8_e4m3
    wq8 = _pair_layout(np.ascontiguousarray(Wq.T), bf)
    wk8 = _pair_layout(np.ascontiguousarray(Wk.T), f8)
    wv8 = _pair_layout(np.ascontiguousarray(Wv.T), f8)
    wo8 = _pair_layout(np.ascontiguousarray(Wo.T), np.float32)
    bq32 = np.ascontiguousarray(bq.reshape(CCH, 128).T).astype(np.float32)
    bk32 = np.ascontiguousarray(bk.reshape(CCH, 128).T).astype(np.float32)
    bv32 = bv.astype(np.float32)
    bo32 = bo.astype(np.float32)
    x2t8 = [_pair_layout(np.ascontiguousarray(x2[b].T), f8) for b in range(B)]
    in_maps = []
    for cid in range(NCORES):
        b, h = cid // 2, cid % 2
        x1s = x1[b, h * QROWS:(h + 1) * QROWS, :]
        in_maps.append({
            "x1t": _pair_layout(np.ascontiguousarray(x1s.T), bf),
            "x2t": x2t8[b],
            "wq8": wq8, "wk8": wk8, "wv8": wv8, "wo8": wo8,
            "bq": bq32, "bk": bk32, "bv": bv32, "bo": bo32,
        })
    return in_maps


LAST_RESULT = None


def kernel(x1, x2, Wq, bq, Wk, bk, Wv, bv, Wo, bo):
    global LAST_RESULT
    nc = get_built()
    in_maps = make_in_maps(x1, x2, Wq, bq, Wk, bk, Wv, bv, Wo, bo)
    trace = bool(os.environ.get("KERNEL_TRACE"))
    res = run_bass_kernel_spmd(nc, in_maps, core_ids=list(range(NCORES)), trace=trace)
    LAST_RESULT = res
    out = np.empty((B, N1, C), dtype=np.float32)
    for cid in range(NCORES):
        b, h = cid // 2, cid % 2
        out[b, h * QROWS:(h + 1) * QROWS, :] = \
            res.results[cid]["out"].reshape(QROWS, C)
    return out
